# revision 5
# baseline (speedup 1.0000x reference)
"""Trainium2 Bass kernel for nn_Attention_Embedding (spatial NxN attention +
channel CxC attention + conv3d(1,1,4) embedding head).

Sharding: 8 cores = 4 samples x 2 halves (split on H). Each core holds its
sample's full q (softmax rows are complete) and produces its own slice of the
final output; no cross-core communication.

v2 vs baseline (123us):
  - exp split across ACT (exact, bf16 out) and DVE (Schraudolph bit-trick:
    i16(round(x*128/ln2 + B)) bitcast to bf16, ~±3.5% per element -- final
    error stays ~1e-3 because beta*conv gain is ~0.02 and the softmax
    num/denom share the same approximated values). Halves the ~75us exp wall.
  - two half-phases over i (slices {0,1} then {2,3}) so half A's
    normalize/conv/relu/transpose/DMA tail overlaps half B's S/exp/AV.
  - S matmuls roll through all 4 PE row-groups (%4) instead of 3.
  - denominator broadcast via DVE stream_shuffle (quadrant broadcast from
    psum) instead of the DRAM round-trip.
  - constants packed into 3 DMAs; q^T replicated host-side (one 1MB DMA,
    no on-chip copies); qTloc_f is a bitcast view of the f32r tile.
  - caF/paF kept in bf16; elementwise tail work on GPSIMD.
"""

import os
import sys

for _p in ("/opt/trn_rl_repo", "/root/.axon_site/_ro/trn_rl_repo"):
    if os.path.isdir(_p) and _p not in sys.path:
        sys.path.insert(0, _p)
        break

import ml_dtypes
import numpy as np

import concourse.bacc as bacc
import concourse.bass as bass
import concourse.mybir as mybir
import concourse.tile as tile
from concourse import bass_utils

B, H, W, D, C = 4, 16, 16, 16, 32
N = H * W * D            # 4096
NL = N // 2              # 2048 rows per core
DO = D - 3               # 13 conv output positions
NCORES = 8
NJC = N // 128           # 32 j-chunks
NIT = NL // 512          # 4 i-slices of 512

f32 = mybir.dt.float32
f32r = mybir.dt.float32r
bf16 = mybir.dt.bfloat16
i16 = mybir.dt.int16
FT = mybir.ActivationFunctionType
ALU = mybir.AluOpType
PSUM = bass.MemorySpace.PSUM

# Schraudolph bf16 exp on DVE: i16(round(x*A16 + B16)) bitcast bf16 ~ exp(x).
LN2 = 0.6931471805599453
A16 = 128.0 / LN2
B16 = 127.0 * 128.0 - 4.46   # magic-c correction balances error to ~±3.5%

# exp tiles per half assigned to DVE (rest go to ACT); 22 tiles per half.
N_DVE_PER_HALF = 10

# packed-constant layouts
PK_WQ, PK_WK, PK_SMALL = 0, 32, 64      # f32 pack: wq/wk rows 0:33, smalls
PKF = 68
PR_WVT, PR_ID32 = 0, 33                 # f32r pack
PRF = 65
PB_WCH, PB_WPOS = 0, 128                # bf16 pack (conv weights, x4 rows)
PBF = 256


def _dve_pattern(n_tiles, n_dve):
    out, acc = [], 0
    for _ in range(n_tiles):
        acc += n_dve
        if acc >= n_tiles:
            acc -= n_tiles
            out.append(True)
        else:
            out.append(False)
    return out


def _emit(tc, nc, t, out_d):
    with (
        tc.tile_pool(name="const", bufs=1) as cp,
        tc.tile_pool(name="work", bufs=1) as wp,
    ):
        # ---- SBUF tiles ----
        qTP_r = cp.tile([128, N], bf16)        # q^T replicated x4 (host-side)
        qTloc_r = cp.tile([C + 1, NL], f32r)   # local q_aug^T (f32 bits)
        qc2_b = cp.tile([128, NJC, 128], bf16)  # [data|ones] x2 AV weights
        qc_f = cp.tile([128, NJC, C + 1], f32)  # gram operand
        pk = cp.tile([128, PKF], f32)
        pkr = cp.tile([C, PRF], f32r)
        pkb = cp.tile([128, PBF], bf16)

        qTloc_f = qTloc_r[0:C, 0:NL].bitcast(f32)
        wq_f = pk[0:C + 1, PK_WQ:PK_WQ + C]
        wk_f = pk[0:C + 1, PK_WK:PK_WK + C]
        bch_v = pk[0:C, PK_SMALL:PK_SMALL + 1]
        bpos_v = pk[0:C, PK_SMALL + 1:PK_SMALL + 2]
        gamma_v = pk[0:C, PK_SMALL + 2:PK_SMALL + 3]
        beta128_v = pk[:, PK_SMALL + 3:PK_SMALL + 4]
        beta_v = pk[0:1, PK_SMALL + 3:PK_SMALL + 4]
        wvT_r = pkr[0:C, PR_WVT:PR_WVT + C + 1]
        id32_r = pkr[0:C, PR_ID32:PR_ID32 + C]
        wch4 = pkb[:, PB_WCH:PB_WCH + 4 * C]
        wpos4 = pkb[:, PB_WPOS:PB_WPOS + 4 * C]

        # ---- input DMAs spread across queues ----
        nc.gpsimd.dma_start(qc_f[:], t["qcf"])     # gram operand first (PE warmup)
        nc.sync.dma_start(qTP_r[:], t["qT"])
        nc.scalar.dma_start(pk[:], t["pk"])
        nc.scalar.dma_start(pkr[:], t["pkr"])
        nc.scalar.dma_start(pkb[:], t["pkb"])
        nc.sync.dma_start(qTloc_r[:], t["qTloc"])
        nc.gpsimd.dma_start(qc2_b[:, :, 0:C], t["qc2d"])
        # trigger the ACT exp table load immediately (~1.3us)
        warm = wp.tile([1, 1], f32)
        nc.scalar.activation(warm[:], beta_v, FT.Exp)
        # qc2 = [data | ones] replicated onto both 64-column halves
        nc.vector.memset(qc2_b[:, :, C:2 * C], 1.0)
        nc.vector.tensor_copy(qc2_b[:, :, 2 * C:4 * C], qc2_b[:, :, 0:2 * C])

        relu_pos = wp.tile([C, NL], f32)
        relu_ch = wp.tile([C, NL], f32)
        sumT = wp.tile([C, NL], f32)
        paF_r = wp.tile([128, NL + 4], bf16)
        caF_r = wp.tile([128, NL + 4], bf16)
        tmp_ca = wp.tile([C, NL], f32)
        out_v = out_d.rearrange("(g kk r) f -> g r kk f", kk=16, r=C)

        # zero the conv-window pads: cols NL..NL+4 (block tail) and the
        # half-A/half-B seam cols 1024..1028 of caF (half A's conv reads 3
        # cols past its range into not-yet-written half-B territory).
        nc.vector.memset(paF_r[:, NL:NL + 4], 0.0)
        nc.vector.memset(caF_r[:, NL:NL + 4], 0.0)
        nc.vector.memset(caF_r[:, NL // 2:NL // 2 + 4], 0.0)

        with tc.tile_pool(name="psAV", bufs=1, space=PSUM) as psAV:
            # ===== small (channel-attention) branch; also the PE HAM warmup,
            # borrowing the avA psum slot before half A's AV first writes it.
            g_ps = psAV.tile([C + 1, C + 1], f32, tag="avA")
            for jc in range(NJC):
                nc.tensor.matmul(
                    g_ps[:], qc_f[:, jc, :], qc_f[:, jc, :],
                    start=(jc == 0), stop=(jc == NJC - 1),
                )
            g_sb = wp.tile([C + 1, C + 1], f32)
            nc.vector.tensor_copy(g_sb[:], g_ps[:])
            # T1 = G @ wk_aug ; energy2 = wq_aug^T @ T1 (partitions=rows of
            # energy2, softmax along free axis)
            t1_ps = psAV.tile([C + 1, C], f32, tag="avA")
            nc.tensor.matmul(t1_ps[:], g_sb[:], wk_f, start=True, stop=True)
            t1_sb = wp.tile([C + 1, C], f32)
            nc.vector.tensor_copy(t1_sb[:], t1_ps[:])
            e2_ps = psAV.tile([C, C], f32, tag="avA")
            nc.tensor.matmul(e2_ps[:], wq_f, t1_sb[:], start=True, stop=True)
            # attn2 = softmax over free; energy2 spans ~[-290, 290]: max-sub
            mx = wp.tile([C, 1], f32)
            nc.vector.reduce_max(mx[:], e2_ps[:], axis=mybir.AxisListType.X)
            nmx = wp.tile([C, 1], f32)
            nc.vector.tensor_scalar_mul(nmx[:], mx[:], -1.0)
            a_sb = wp.tile([C, C], f32)
            nc.scalar.activation(a_sb[:], e2_ps[:], FT.Exp, bias=nmx[:])
            sm = wp.tile([C, 1], f32)
            nc.vector.reduce_sum(sm[:], a_sb[:], axis=mybir.AxisListType.X)
            rc = wp.tile([C, 1], f32)
            nc.vector.reciprocal(rc[:], sm[:])
            a_n = wp.tile([C, C], f32r)
            nc.vector.tensor_scalar_mul(a_n[:], a_sb[:], rc[:])
            at_ps = psAV.tile([C, C], f32, tag="avA")
            nc.tensor.matmul(at_ps[:], a_n[:], id32_r, start=True, stop=True)
            at_r = wp.tile([C, C], f32r)
            nc.vector.tensor_copy(at_r[:], at_ps[:])
            # wpa = wv_aug @ attn2^T, so pa_T = wpa @ q_loc_aug^T directly
            wpa_ps = psAV.tile([C + 1, C], f32, tag="avA")
            nc.tensor.matmul(wpa_ps[:], wvT_r, at_r[:], start=True, stop=True)
            wpa_r = wp.tile([C + 1, C], f32r)
            nc.vector.tensor_copy(wpa_r[:], wpa_ps[:])
            for g in range(NIT):
                pa_ps = psAV.tile([C, 512], f32, tag="avA")
                nc.tensor.matmul(
                    pa_ps[:], wpa_r[:], qTloc_r[:, g * 512:(g + 1) * 512],
                    start=True, stop=True,
                )
                nc.vector.scalar_tensor_tensor(
                    paF_r[0:C, g * 512:(g + 1) * 512], pa_ps[:], gamma_v,
                    qTloc_f[:, g * 512:(g + 1) * 512],
                    op0=ALU.mult, op1=ALU.add,
                )
            # replicate onto row-groups 1..3 for the row-tiled conv
            nc.gpsimd.dma_start(paF_r[C:2 * C, :], paF_r[0:C, :])
            nc.sync.dma_start(paF_r[2 * C:3 * C, :], paF_r[0:C, :])
            nc.scalar.dma_start(paF_r[3 * C:4 * C, :], paF_r[0:C, :])

            av0 = psAV.tile([128, 512], f32, tag="avA")  # half A: slices 0,1
            av1 = psAV.tile([128, 512], f32, tag="avB")  # half B: slices 2,3

            # ================= big branch: two half-phases =================
            s_roll = [0]   # rolling PE row-group for S matmuls

            def emit_av(av_t, jc, pt_pair):
                for p in range(2):
                    pt_ap, off = pt_pair[p]
                    nc.tensor.matmul(
                        av_t[64 * p:64 * p + 64, :],
                        qc2_b[:, jc, 64 * p:64 * p + 64],
                        pt_ap[:, off:off + 512],
                        start=(jc == 0), stop=(jc == NJC - 1),
                        tile_position=(0, 64 * p), skip_group_check=True,
                    )

            def emit_half(h, av_t, psS, ptp, dve_tiles, tail_hooks):
                """S + exp + AV for slices (2h, 2h+1); tail_hooks fire after
                the given tile index to interleave prior-half tail work."""
                NSL = NJC * 2
                NTL = (NSL + 2) // 3
                slice_loc = {}
                done_jc = 0
                hooks = dict(tail_hooks)
                for tl_i in range(NTL):
                    idx0 = tl_i * 3
                    nsl = min(3, NSL - idx0)
                    s_ps = psS.tile([128, 1536], f32, tag="s")
                    for r in range(nsl):
                        jc, s = divmod(idx0 + r, 2)
                        sg = 2 * h + s
                        rp = s_roll[0] & 3
                        s_roll[0] += 1
                        nc.tensor.matmul(
                            s_ps[:, r * 512:(r + 1) * 512],
                            qTP_r[32 * rp:32 * rp + C, jc * 128:(jc + 1) * 128],
                            qTP_r[32 * rp:32 * rp + C, sg * 512:(sg + 1) * 512],
                            start=True, stop=True,
                            tile_position=(32 * rp, 0), skip_group_check=True,
                        )
                    if dve_tiles[tl_i]:
                        pti = ptp.tile([128, 1536], i16, tag="pt")
                        nc.vector.tensor_scalar(
                            pti[:, 0:nsl * 512], s_ps[:, 0:nsl * 512],
                            A16, B16, op0=ALU.mult, op1=ALU.add,
                        )
                        pt_ap = pti[:].bitcast(bf16)
                    else:
                        ptt = ptp.tile([128, 1536], bf16, tag="pt")
                        nc.scalar.activation(
                            ptt[:, 0:nsl * 512], s_ps[:, 0:nsl * 512], FT.Exp
                        )
                        pt_ap = ptt[:]
                    for r in range(nsl):
                        slice_loc[idx0 + r] = (pt_ap, r * 512)
                    while done_jc < NJC and (done_jc + 1) * 2 <= idx0 + nsl:
                        emit_av(av_t, done_jc,
                                [slice_loc[done_jc * 2 + s] for s in range(2)])
                        done_jc += 1
                    if tl_i in hooks:
                        hooks.pop(tl_i)()
                for k in sorted(hooks):
                    hooks.pop(k)()

            def emit_norm(h, av_t):
                """caF[0:C, h*1024:+1024] = beta*ca/denom + q_loc, then
                replicate to row-groups 1..3."""
                c0 = h * 1024
                shf = wp.tile([128, 512], f32, tag=f"shf{h}")
                nc.vector.stream_shuffle(shf[:], av_t[:], [0] * 32)
                rec = wp.tile([128, 512], f32, tag=f"rec{h}")
                nc.vector.reciprocal_approx_fast(rec[:], shf[:])
                recB = wp.tile([128, 512], f32, tag=f"recB{h}")
                nc.vector.tensor_scalar_mul(recB[:], rec[:], beta128_v)
                for p in range(2):
                    # tmp = ca * (beta/denom)
                    nc.vector.tensor_mul(
                        tmp_ca[:, c0 + p * 512:c0 + (p + 1) * 512],
                        av_t[64 * p:64 * p + C, :],
                        recB[64 * p + C:64 * p + 2 * C, :],
                    )
                nc.gpsimd.tensor_add(
                    caF_r[0:C, c0:c0 + 1024],
                    tmp_ca[:, c0:c0 + 1024], qTloc_f[:, c0:c0 + 1024],
                )
                nc.gpsimd.dma_start(caF_r[C:2 * C, c0:c0 + 1024], caF_r[0:C, c0:c0 + 1024])
                nc.sync.dma_start(caF_r[2 * C:3 * C, c0:c0 + 1024], caF_r[0:C, c0:c0 + 1024])
                nc.scalar.dma_start(caF_r[3 * C:4 * C, c0:c0 + 1024], caF_r[0:C, c0:c0 + 1024])

            def conv_chunk(pool, tag, w4, x4, bias_v, relu_out, ci):
                """one [C, 512] conv chunk: relu(b + sum_k w_k x[.,n+k])."""
                rp = ci & 3
                cv = pool.tile([C, 512], f32, tag=tag)
                for k in range(4):
                    nc.tensor.matmul(
                        cv[:],
                        w4[32 * rp:32 * rp + C, k * C:(k + 1) * C],
                        x4[32 * rp:32 * rp + C, ci * 512 + k:ci * 512 + k + 512],
                        start=(k == 0), stop=(k == 3),
                        tile_position=(32 * rp, 0), skip_group_check=True,
                    )
                nc.scalar.activation(
                    relu_out[:, ci * 512:(ci + 1) * 512], cv[:],
                    FT.Relu, bias=bias_v,
                )

            def emit_convs(h, pool, tag):
                for ci in (2 * h, 2 * h + 1):
                    conv_chunk(pool, tag, wch4, caF_r, bch_v, relu_ch, ci)
                    conv_chunk(pool, tag, wpos4, paF_r, bpos_v, relu_pos, ci)

            def emit_outs(h):
                for ci in (2 * h, 2 * h + 1):
                    nc.gpsimd.tensor_add(
                        sumT[:, ci * 512:(ci + 1) * 512],
                        relu_ch[:, ci * 512:(ci + 1) * 512],
                        relu_pos[:, ci * 512:(ci + 1) * 512],
                    )
                    tb = wp.tile([C, 512], f32, tag=f"ob{ci & 1}")
                    nc.vector.transpose(tb[:], sumT[:, ci * 512:(ci + 1) * 512])
                    eng = nc.sync if ci % 2 == 0 else nc.gpsimd
                    eng.dma_start(
                        out_v[ci],
                        tb[:].rearrange("r (kk f) -> r kk f", kk=16),
                    )

            patA = _dve_pattern(22, N_DVE_PER_HALF)
            patB = _dve_pattern(22, N_DVE_PER_HALF)
            with (
                tc.tile_pool(name="psS", bufs=2, space=PSUM) as psS,
                tc.tile_pool(name="ptp", bufs=8) as ptp,
            ):
                emit_half(0, av0, psS, ptp, patA, {})
                emit_half(
                    1, av1, psS, ptp, patB,
                    {
                        0: lambda: emit_norm(0, av0),
                        5: lambda: emit_convs(0, psAV, "avA"),
                        11: lambda: emit_outs(0),
                    },
                )
                emit_norm(1, av1)
            # psS closed: 6 banks free for a pipelined final conv
            with tc.tile_pool(name="psC2", bufs=2, space=PSUM) as psC2:
                emit_convs(1, psC2, "cv")
                emit_outs(1)


def _build():
    nc = bacc.Bacc("TRN2", target_bir_lowering=False, debug=False)
    t = {}

    def din(name, shape, dt):
        t[name] = nc.dram_tensor(name, shape, dt, kind="ExternalInput").ap()

    din("qT", [128, N], bf16)
    din("qTloc", [C + 1, NL], f32r)
    din("qc2d", [128, NJC, C], bf16)
    din("qcf", [128, NJC, C + 1], f32)
    din("pk", [128, PKF], f32)
    din("pkr", [C, PRF], f32r)
    din("pkb", [128, PBF], bf16)
    out_d = nc.dram_tensor("out", [NL, C], f32, kind="ExternalOutput").ap()

    with tile.TileContext(nc) as tc:
        _emit(tc, nc, t, out_d)
    nc.compile()
    return nc


_NC = None


def _get_nc():
    global _NC
    if _NC is None:
        _NC = _build()
    return _NC


def _prepare_in_maps(inputs):
    x = np.asarray(inputs["inputs"], np.float32)
    beta = np.asarray(inputs["beta"], np.float32)
    gamma = np.asarray(inputs["gamma"], np.float32)
    wq_aug = np.concatenate(
        [np.asarray(inputs["wq"], np.float32), np.asarray(inputs["bq"], np.float32)[None, :]], 0
    )
    wk_aug = np.concatenate(
        [np.asarray(inputs["wk"], np.float32), np.asarray(inputs["bk"], np.float32)[None, :]], 0
    )
    wv_aug = np.concatenate(
        [np.asarray(inputs["wv"], np.float32), np.asarray(inputs["bv"], np.float32)[None, :]], 0
    )
    pk = np.zeros((128, PKF), np.float32)
    pk[0:C + 1, PK_WQ:PK_WQ + C] = wq_aug
    pk[0:C + 1, PK_WK:PK_WK + C] = wk_aug
    pk[0:C, PK_SMALL] = np.asarray(inputs["b_ch"], np.float32)
    pk[0:C, PK_SMALL + 1] = np.asarray(inputs["b_pos"], np.float32)
    pk[0:C, PK_SMALL + 2] = gamma[0]
    pk[0:128, PK_SMALL + 3] = beta[0]
    pkr = np.zeros((C, PRF), np.float32)
    pkr[0:C, PR_WVT:PR_WVT + C + 1] = wv_aug.T
    pkr[0:C, PR_ID32:PR_ID32 + C] = np.eye(C, dtype=np.float32)
    pkb = np.zeros((128, PBF), np.float32)
    pkb[:, PB_WCH:PB_WCH + 4 * C] = np.tile(
        np.asarray(inputs["w_ch"], np.float32).reshape(4, C, C).transpose(1, 0, 2).reshape(C, 4 * C),
        (4, 1),
    )
    pkb[:, PB_WPOS:PB_WPOS + 4 * C] = np.tile(
        np.asarray(inputs["w_pos"], np.float32).reshape(4, C, C).transpose(1, 0, 2).reshape(C, 4 * C),
        (4, 1),
    )
    pkb = pkb.astype(ml_dtypes.bfloat16)

    in_maps = []
    for core in range(NCORES):
        b, s = core // 2, core % 2
        qs = x[b].reshape(N, C)
        # local-half-first column permutation: S_T rhs slices [0, NL) are the
        # core's own rows; softmax sums over all j are order-invariant.
        q = np.concatenate([qs[s * NL:(s + 1) * NL], qs[(1 - s) * NL:(2 - s) * NL]])
        q_aug = np.concatenate([q, np.ones((N, 1), np.float32)], 1)
        qloc_aug = q_aug[:NL]
        qc = np.ascontiguousarray(q_aug.reshape(NJC, 128, C + 1).transpose(1, 0, 2))
        qT_b = np.ascontiguousarray(q.T).astype(ml_dtypes.bfloat16)
        m = {
            "qT": np.ascontiguousarray(np.tile(qT_b, (4, 1))),
            "qTloc": np.ascontiguousarray(qloc_aug.T),
            "qc2d": np.ascontiguousarray(qc[:, :, :C]).astype(ml_dtypes.bfloat16),
            "qcf": qc,
            "pk": pk,
            "pkr": pkr,
            "pkb": pkb,
        }
        in_maps.append(m)
    return in_maps


def _run(inputs, trace=False):
    nc = _get_nc()
    in_maps = _prepare_in_maps(inputs)
    res = bass_utils.run_bass_kernel_spmd(
        nc, in_maps, core_ids=list(range(NCORES)), trace=trace
    )
    out = np.empty((B, H, W, DO, C), np.float32)
    for core in range(NCORES):
        b, s = core // 2, core % 2
        full = res.results[core]["out"].reshape(8, W, D, C)
        out[b, s * 8:(s + 1) * 8] = full[:, :, :DO, :]
    return out, res


def kernel(**inputs):
    out, _ = _run(inputs, trace=False)
    return out


# revision 7
# speedup vs baseline: 1.2616x; 1.2616x over previous
"""Trainium2 Bass kernel for nn_Attention_Embedding (spatial NxN attention +
channel CxC attention + conv3d(1,1,4) embedding head).

Sharding: 8 cores = 4 samples x 2 halves (split on H). Each core holds its
sample's full q (softmax rows are complete) and produces its own slice of the
final output; no cross-core communication.

v3 (from the v2 trace: PE-bound at the HAM-throttled 1.2GHz clock, ~300ns
fixed cost per matmul, S/AV subarray-conflict serialization):
  - exp split across ACT (exact, bf16 out) and DVE (Schraudolph bit-trick:
    i16(round(x*128/ln2 + B)) bitcast to bf16, ~±3.5% per element).
  - S tiles are [128,1024] = one j-chunk x both slices of the half; AV for
    jc is emitted one tile late so the PE never stalls waiting for exp.
  - conv3d via true im2col: the 4 row-group replicas of caF/paF are SHIFTED
    copies (replica k holds x[c, n+k]) so each conv chunk is ONE K=128
    matmul instead of 4 accumulating K=32 matmuls.
  - gram (small-branch) matmuls interleaved into half A as PE bubble filler.
  - two half-phases over i so half A's tail overlaps half B's S/exp/AV.
  - denominator broadcast via DVE stream_shuffle; tail elementwise on GPSIMD.
"""

import os
import sys

for _p in ("/opt/trn_rl_repo", "/root/.axon_site/_ro/trn_rl_repo"):
    if os.path.isdir(_p) and _p not in sys.path:
        sys.path.insert(0, _p)
        break

import ml_dtypes
import numpy as np

import concourse.bacc as bacc
import concourse.bass as bass
import concourse.mybir as mybir
import concourse.tile as tile
from concourse import bass_utils

B, H, W, D, C = 4, 16, 16, 16, 32
N = H * W * D            # 4096
NL = N // 2              # 2048 rows per core
DO = D - 3               # 13 conv output positions
NCORES = 8
NJC = N // 128           # 32 j-chunks

f32 = mybir.dt.float32
f32r = mybir.dt.float32r
bf16 = mybir.dt.bfloat16
i16 = mybir.dt.int16
FT = mybir.ActivationFunctionType
ALU = mybir.AluOpType
PSUM = bass.MemorySpace.PSUM

# Schraudolph bf16 exp on DVE: i16(round(x*A16 + B16)) bitcast bf16 ~ exp(x).
LN2 = 0.6931471805599453
A16 = 128.0 / LN2
B16 = 127.0 * 128.0 - 4.46   # magic-c correction balances error to ~±3.5%

N_DVE_PER_HALF = 15          # of 32 exp tiles per half on DVE (rest ACT)

# packed-constant layouts
PK_WQ, PK_WK, PK_SMALL = 0, 32, 64      # f32 pack: wq/wk rows 0:33, smalls
PKF = 68
PR_WVT, PR_ID32 = 0, 33                 # f32r pack
PRF = 65
PB_WCH, PB_WPOS = 0, 32                 # bf16 pack: stacked im2col weights
PBF = 64


def _dve_pattern(n_tiles, n_dve):
    out, acc = [], 0
    for _ in range(n_tiles):
        acc += n_dve
        if acc >= n_tiles:
            acc -= n_tiles
            out.append(True)
        else:
            out.append(False)
    return out


def _emit(tc, nc, t, out_d):
    with (
        tc.tile_pool(name="const", bufs=1) as cp,
        tc.tile_pool(name="work", bufs=1) as wp,
    ):
        # ---- SBUF tiles ----
        qTP_r = cp.tile([128, N], bf16)        # q^T replicated x4 (host-side)
        qTloc_r = cp.tile([C + 1, NL], f32r)   # local q_aug^T (f32 bits)
        qc2_b = cp.tile([128, NJC, 128], bf16)  # [data|ones] x2 AV weights
        qc_f = cp.tile([128, NJC, C + 1], f32)  # gram operand
        pk = cp.tile([128, PKF], f32)
        pkr = cp.tile([C, PRF], f32r)
        pkb = cp.tile([128, PBF], bf16)

        qTloc_f = qTloc_r[0:C, 0:NL].bitcast(f32)
        wq_f = pk[0:C + 1, PK_WQ:PK_WQ + C]
        wk_f = pk[0:C + 1, PK_WK:PK_WK + C]
        bch_v = pk[0:C, PK_SMALL:PK_SMALL + 1]
        bpos_v = pk[0:C, PK_SMALL + 1:PK_SMALL + 2]
        gamma_v = pk[0:C, PK_SMALL + 2:PK_SMALL + 3]
        beta128_v = pk[:, PK_SMALL + 3:PK_SMALL + 4]
        beta_v = pk[0:1, PK_SMALL + 3:PK_SMALL + 4]
        wvT_r = pkr[0:C, PR_WVT:PR_WVT + C + 1]
        id32_r = pkr[0:C, PR_ID32:PR_ID32 + C]
        wch_st = pkb[:, PB_WCH:PB_WCH + C]
        wpos_st = pkb[:, PB_WPOS:PB_WPOS + C]

        # ---- input DMAs spread across queues ----
        nc.gpsimd.dma_start(qc_f[:], t["qcf"])     # gram operand first
        nc.sync.dma_start(qTP_r[:], t["qT"])
        nc.scalar.dma_start(pk[:], t["pk"])
        nc.scalar.dma_start(pkr[:], t["pkr"])
        nc.scalar.dma_start(pkb[:], t["pkb"])
        nc.sync.dma_start(qTloc_r[:], t["qTloc"])
        nc.gpsimd.dma_start(qc2_b[:, :, 0:C], t["qc2d"])
        # trigger the ACT exp table load immediately (~1.3us)
        warm = wp.tile([1, 1], f32)
        nc.scalar.activation(warm[:], beta_v, FT.Exp)
        # qc2 = [data | ones] replicated onto both 64-column halves
        nc.vector.memset(qc2_b[:, :, C:2 * C], 1.0)
        nc.vector.tensor_copy(qc2_b[:, :, 2 * C:4 * C], qc2_b[:, :, 0:2 * C])

        relu_pos = wp.tile([C, NL], f32)
        relu_ch = wp.tile([C, NL], f32)
        sumT = wp.tile([C, NL], f32)
        paF_r = wp.tile([128, NL + 4], bf16)
        caF_r = wp.tile([128, NL + 4], bf16)
        tmp_ca = wp.tile([C, NL], f32)
        out_v = out_d.rearrange("(g kk r) f -> g r kk f", kk=16, r=C)

        # zero the conv-window pads: cols NL..NL+4 (block tail) and the
        # half-A/half-B seam cols 1024..1028 of caF (half A's shifted
        # replicas read 3 cols into not-yet-written half-B territory).
        nc.vector.memset(paF_r[:, NL:NL + 4], 0.0)
        nc.vector.memset(caF_r[:, NL:NL + 4], 0.0)
        nc.vector.memset(caF_r[:, NL // 2:NL // 2 + 4], 0.0)

        with tc.tile_pool(name="psAV", bufs=1, space=PSUM) as psAV:
            av0 = psAV.tile([128, 512], f32, tag="avA")  # half A: slices 0,1
            av1 = psAV.tile([128, 512], f32, tag="avB")  # half B: slices 2,3
            g_ps = psAV.tile([C + 1, C + 1], f32, tag="g")

            def emit_gram(j0, j1):
                for jc in range(j0, j1):
                    nc.tensor.matmul(
                        g_ps[:], qc_f[:, jc, :], qc_f[:, jc, :],
                        start=(jc == 0), stop=(jc == NJC - 1),
                    )

            def emit_small1():
                """G -> attn2 -> wpa (PE + tiny DVE/ACT ops)."""
                g_sb = wp.tile([C + 1, C + 1], f32)
                nc.vector.tensor_copy(g_sb[:], g_ps[:])
                t1_ps = psAV.tile([C + 1, C], f32, tag="g")
                nc.tensor.matmul(t1_ps[:], g_sb[:], wk_f, start=True, stop=True)
                t1_sb = wp.tile([C + 1, C], f32)
                nc.vector.tensor_copy(t1_sb[:], t1_ps[:])
                e2_ps = psAV.tile([C, C], f32, tag="g")
                nc.tensor.matmul(e2_ps[:], wq_f, t1_sb[:], start=True, stop=True)
                mx = wp.tile([C, 1], f32)
                nc.vector.reduce_max(mx[:], e2_ps[:], axis=mybir.AxisListType.X)
                nmx = wp.tile([C, 1], f32)
                nc.vector.tensor_scalar_mul(nmx[:], mx[:], -1.0)
                a_sb = wp.tile([C, C], f32)
                nc.scalar.activation(a_sb[:], e2_ps[:], FT.Exp, bias=nmx[:])
                sm = wp.tile([C, 1], f32)
                nc.vector.reduce_sum(sm[:], a_sb[:], axis=mybir.AxisListType.X)
                rc = wp.tile([C, 1], f32)
                nc.vector.reciprocal(rc[:], sm[:])
                a_n = wp.tile([C, C], f32r)
                nc.vector.tensor_scalar_mul(a_n[:], a_sb[:], rc[:])
                at_ps = psAV.tile([C, C], f32, tag="g")
                nc.tensor.matmul(at_ps[:], a_n[:], id32_r, start=True, stop=True)
                at_r = wp.tile([C, C], f32r)
                nc.vector.tensor_copy(at_r[:], at_ps[:])
                wpa_ps = psAV.tile([C + 1, C], f32, tag="g")
                nc.tensor.matmul(wpa_ps[:], wvT_r, at_r[:], start=True, stop=True)
                wpa_r = wp.tile([C + 1, C], f32r)
                nc.vector.tensor_copy(wpa_r[:], wpa_ps[:])
                return wpa_r

            def emit_small2(wpa_r):
                """pa branch + shifted replicas of paF."""
                for g in range(4):
                    pa_ps = psAV.tile([C, 512], f32, tag="g")
                    nc.tensor.matmul(
                        pa_ps[:], wpa_r[:], qTloc_r[:, g * 512:(g + 1) * 512],
                        start=True, stop=True,
                    )
                    nc.vector.scalar_tensor_tensor(
                        paF_r[0:C, g * 512:(g + 1) * 512], pa_ps[:], gamma_v,
                        qTloc_f[:, g * 512:(g + 1) * 512],
                        op0=ALU.mult, op1=ALU.add,
                    )
                for k in (1, 2, 3):
                    eng = (nc.gpsimd, nc.sync, nc.scalar)[k - 1]
                    eng.dma_start(
                        paF_r[32 * k:32 * k + C, 0:NL],
                        paF_r[0:C, k:k + NL],
                    )

            # ================= big branch: two half-phases =================
            s_roll = [0]   # rolling PE row-group for S matmuls

            def emit_av(av_t, jc, pt_ap):
                for p in range(2):
                    nc.tensor.matmul(
                        av_t[64 * p:64 * p + 64, :],
                        qc2_b[:, jc, 64 * p:64 * p + 64],
                        pt_ap[:, 512 * p:512 * p + 512],
                        start=(jc == 0), stop=(jc == NJC - 1),
                        tile_position=(0, 64 * p), skip_group_check=True,
                    )

            def emit_half(h, av_t, psS, ptp, dve_tiles, hooks):
                """S + exp + AV for slices (2h, 2h+1). One tile per jc; AV
                for jc is emitted after tile jc+1's exp (PE slack). hooks
                fire after the given tile index."""
                hooks = dict(hooks)
                pt_tiles = {}
                for jc in range(NJC):
                    s_ps = psS.tile([128, 1024], f32, tag="s")
                    for s in range(2):
                        rp = s_roll[0] & 3
                        s_roll[0] += 1
                        cg = (2 * h + s) * 512
                        nc.tensor.matmul(
                            s_ps[:, s * 512:(s + 1) * 512],
                            qTP_r[32 * rp:32 * rp + C, jc * 128:(jc + 1) * 128],
                            qTP_r[32 * rp:32 * rp + C, cg:cg + 512],
                            start=True, stop=True,
                            tile_position=(32 * rp, 0), skip_group_check=True,
                        )
                    if dve_tiles[jc]:
                        pti = ptp.tile([128, 1024], i16, tag="pt")
                        nc.vector.tensor_scalar(
                            pti[:], s_ps[:], A16, B16, op0=ALU.mult, op1=ALU.add,
                        )
                        pt_tiles[jc] = pti[:].bitcast(bf16)
                    else:
                        ptt = ptp.tile([128, 1024], bf16, tag="pt")
                        nc.scalar.activation(ptt[:], s_ps[:], FT.Exp)
                        pt_tiles[jc] = ptt[:]
                    if jc >= 1:
                        emit_av(av_t, jc - 1, pt_tiles.pop(jc - 1))
                    if jc in hooks:
                        hooks.pop(jc)()
                emit_av(av_t, NJC - 1, pt_tiles.pop(NJC - 1))
                for k in sorted(hooks):
                    hooks.pop(k)()

            def emit_norm(h, av_t):
                """caF[0:C, h*1024:+1024] = beta*ca/denom + q_loc, then the
                shifted replicas onto row-groups 1..3."""
                c0 = h * 1024
                shf = wp.tile([128, 512], f32, tag=f"shf{h}")
                nc.vector.stream_shuffle(shf[:], av_t[:], [0] * 32)
                rec = wp.tile([128, 512], f32, tag=f"rec{h}")
                nc.vector.reciprocal_approx_fast(rec[:], shf[:])
                recB = wp.tile([128, 512], f32, tag=f"recB{h}")
                nc.vector.tensor_scalar_mul(recB[:], rec[:], beta128_v)
                for p in range(2):
                    nc.vector.tensor_mul(
                        tmp_ca[:, c0 + p * 512:c0 + (p + 1) * 512],
                        av_t[64 * p:64 * p + C, :],
                        recB[64 * p + C:64 * p + 2 * C, :],
                    )
                nc.gpsimd.tensor_add(
                    caF_r[0:C, c0:c0 + 1024],
                    tmp_ca[:, c0:c0 + 1024], qTloc_f[:, c0:c0 + 1024],
                )
                for k in (1, 2, 3):
                    eng = (nc.gpsimd, nc.sync, nc.scalar)[k - 1]
                    eng.dma_start(
                        caF_r[32 * k:32 * k + C, c0:c0 + 1024],
                        caF_r[0:C, c0 + k:c0 + k + 1024],
                    )

            def conv_chunk(pool, tag, wst, x2t, bias_v, relu_out, cu):
                """one [C, 512] conv chunk: single K=128 im2col matmul."""
                cv = pool.tile([C, 512], f32, tag=tag)
                nc.tensor.matmul(
                    cv[:], wst, x2t[:, cu * 512:cu * 512 + 512],
                    start=True, stop=True,
                )
                nc.scalar.activation(
                    relu_out[:, cu * 512:(cu + 1) * 512], cv[:],
                    FT.Relu, bias=bias_v,
                )

            def emit_convs(h, pool, tags):
                for i, cu in enumerate((2 * h, 2 * h + 1)):
                    conv_chunk(pool, tags[i % len(tags)], wch_st, caF_r,
                               bch_v, relu_ch, cu)
                    conv_chunk(pool, tags[(i + 1) % len(tags)], wpos_st, paF_r,
                               bpos_v, relu_pos, cu)

            def emit_outs(h):
                for cu in (2 * h, 2 * h + 1):
                    nc.gpsimd.tensor_add(
                        sumT[:, cu * 512:(cu + 1) * 512],
                        relu_ch[:, cu * 512:(cu + 1) * 512],
                        relu_pos[:, cu * 512:(cu + 1) * 512],
                    )
                    tb = wp.tile([C, 512], f32, tag=f"ob{cu & 1}")
                    nc.vector.transpose(tb[:], sumT[:, cu * 512:(cu + 1) * 512])
                    eng = nc.sync if cu % 2 == 0 else nc.gpsimd
                    eng.dma_start(
                        out_v[cu],
                        tb[:].rearrange("r (kk f) -> r kk f", kk=16),
                    )

            patA = _dve_pattern(NJC, N_DVE_PER_HALF)
            patB = _dve_pattern(NJC, N_DVE_PER_HALF)
            wpa_box = {}
            with (
                tc.tile_pool(name="psS", bufs=2, space=PSUM) as psS,
                tc.tile_pool(name="ptp", bufs=8) as ptp,
            ):
                # gram: first chunks fill the head DMA wait; the rest are PE
                # bubble filler between half-A tiles.
                emit_gram(0, 8)
                hooksA = {
                    1: lambda: emit_gram(8, 14),
                    3: lambda: emit_gram(14, 20),
                    5: lambda: emit_gram(20, 26),
                    7: lambda: emit_gram(26, 32),
                    9: lambda: wpa_box.__setitem__("w", emit_small1()),
                    11: lambda: emit_small2(wpa_box["w"]),
                }
                emit_half(0, av0, psS, ptp, patA, hooksA)
                hooksB = {
                    0: lambda: emit_norm(0, av0),
                    6: lambda: emit_convs(0, psAV, ["g", "g2"]),
                    14: lambda: emit_outs(0),
                }
                emit_half(1, av1, psS, ptp, patB, hooksB)
                emit_norm(1, av1)
            # psS closed: banks free for a pipelined final conv
            with tc.tile_pool(name="psC2", bufs=2, space=PSUM) as psC2:
                emit_convs(1, psC2, ["cv"])
                emit_outs(1)


def _build():
    nc = bacc.Bacc("TRN2", target_bir_lowering=False, debug=False)
    t = {}

    def din(name, shape, dt):
        t[name] = nc.dram_tensor(name, shape, dt, kind="ExternalInput").ap()

    din("qT", [128, N], bf16)
    din("qTloc", [C + 1, NL], f32r)
    din("qc2d", [128, NJC, C], bf16)
    din("qcf", [128, NJC, C + 1], f32)
    din("pk", [128, PKF], f32)
    din("pkr", [C, PRF], f32r)
    din("pkb", [128, PBF], bf16)
    out_d = nc.dram_tensor("out", [NL, C], f32, kind="ExternalOutput").ap()

    with tile.TileContext(nc) as tc:
        _emit(tc, nc, t, out_d)
    nc.compile()
    return nc


_NC = None


def _get_nc():
    global _NC
    if _NC is None:
        _NC = _build()
    return _NC


def _prepare_in_maps(inputs):
    x = np.asarray(inputs["inputs"], np.float32)
    beta = np.asarray(inputs["beta"], np.float32)
    gamma = np.asarray(inputs["gamma"], np.float32)
    wq_aug = np.concatenate(
        [np.asarray(inputs["wq"], np.float32), np.asarray(inputs["bq"], np.float32)[None, :]], 0
    )
    wk_aug = np.concatenate(
        [np.asarray(inputs["wk"], np.float32), np.asarray(inputs["bk"], np.float32)[None, :]], 0
    )
    wv_aug = np.concatenate(
        [np.asarray(inputs["wv"], np.float32), np.asarray(inputs["bv"], np.float32)[None, :]], 0
    )
    pk = np.zeros((128, PKF), np.float32)
    pk[0:C + 1, PK_WQ:PK_WQ + C] = wq_aug
    pk[0:C + 1, PK_WK:PK_WK + C] = wk_aug
    pk[0:C, PK_SMALL] = np.asarray(inputs["b_ch"], np.float32)
    pk[0:C, PK_SMALL + 1] = np.asarray(inputs["b_pos"], np.float32)
    pk[0:C, PK_SMALL + 2] = gamma[0]
    pk[0:128, PK_SMALL + 3] = beta[0]
    pkr = np.zeros((C, PRF), np.float32)
    pkr[0:C, PR_WVT:PR_WVT + C + 1] = wv_aug.T
    pkr[0:C, PR_ID32:PR_ID32 + C] = np.eye(C, dtype=np.float32)
    pkb = np.zeros((128, PBF), np.float32)
    pkb[:, PB_WCH:PB_WCH + C] = np.asarray(inputs["w_ch"], np.float32).reshape(4 * C, C)
    pkb[:, PB_WPOS:PB_WPOS + C] = np.asarray(inputs["w_pos"], np.float32).reshape(4 * C, C)
    pkb = pkb.astype(ml_dtypes.bfloat16)

    in_maps = []
    for core in range(NCORES):
        b, s = core // 2, core % 2
        qs = x[b].reshape(N, C)
        # local-half-first column permutation: S_T rhs slices [0, NL) are the
        # core's own rows; softmax sums over all j are order-invariant.
        q = np.concatenate([qs[s * NL:(s + 1) * NL], qs[(1 - s) * NL:(2 - s) * NL]])
        q_aug = np.concatenate([q, np.ones((N, 1), np.float32)], 1)
        qloc_aug = q_aug[:NL]
        qc = np.ascontiguousarray(q_aug.reshape(NJC, 128, C + 1).transpose(1, 0, 2))
        qT_b = np.ascontiguousarray(q.T).astype(ml_dtypes.bfloat16)
        m = {
            "qT": np.ascontiguousarray(np.tile(qT_b, (4, 1))),
            "qTloc": np.ascontiguousarray(qloc_aug.T),
            "qc2d": np.ascontiguousarray(qc[:, :, :C]).astype(ml_dtypes.bfloat16),
            "qcf": qc,
            "pk": pk,
            "pkr": pkr,
            "pkb": pkb,
        }
        in_maps.append(m)
    return in_maps


def _run(inputs, trace=False):
    nc = _get_nc()
    in_maps = _prepare_in_maps(inputs)
    res = bass_utils.run_bass_kernel_spmd(
        nc, in_maps, core_ids=list(range(NCORES)), trace=trace
    )
    out = np.empty((B, H, W, DO, C), np.float32)
    for core in range(NCORES):
        b, s = core // 2, core % 2
        full = res.results[core]["out"].reshape(8, W, D, C)
        out[b, s * 8:(s + 1) * 8] = full[:, :, :DO, :]
    return out, res


def kernel(**inputs):
    out, _ = _run(inputs, trace=False)
    return out


# revision 17
# speedup vs baseline: 1.3946x; 1.1054x over previous
"""Trainium2 Bass kernel for nn_Attention_Embedding (spatial NxN attention +
channel CxC attention + conv3d(1,1,4) embedding head).

Sharding: 8 cores = 4 samples x 2 halves (split on H). Each core holds its
sample's full q (softmax rows are complete) and produces its own slice of the
final output; no cross-core communication.

v3 (from the v2 trace: PE-bound at the HAM-throttled 1.2GHz clock, ~300ns
fixed cost per matmul, S/AV subarray-conflict serialization):
  - exp split across ACT (exact, bf16 out) and DVE (Schraudolph bit-trick:
    i16(round(x*128/ln2 + B)) bitcast to bf16, ~±3.5% per element).
  - S tiles are [128,1024] = one j-chunk x both slices of the half; AV for
    jc is emitted one tile late so the PE never stalls waiting for exp.
  - conv3d via true im2col: the 4 row-group replicas of caF/paF are SHIFTED
    copies (replica k holds x[c, n+k]) so each conv chunk is ONE K=128
    matmul instead of 4 accumulating K=32 matmuls.
  - gram (small-branch) matmuls interleaved into half A as PE bubble filler.
  - two half-phases over i so half A's tail overlaps half B's S/exp/AV.
  - denominator broadcast via DVE stream_shuffle; tail elementwise on GPSIMD.
"""

import os
import sys

for _p in ("/opt/trn_rl_repo", "/root/.axon_site/_ro/trn_rl_repo"):
    if os.path.isdir(_p) and _p not in sys.path:
        sys.path.insert(0, _p)
        break

import ml_dtypes
import numpy as np

import concourse.bacc as bacc
import concourse.bass as bass
import concourse.mybir as mybir
import concourse.tile as tile
from concourse import bass_utils

B, H, W, D, C = 4, 16, 16, 16, 32
N = H * W * D            # 4096
NL = N // 2              # 2048 rows per core
DO = D - 3               # 13 conv output positions
NCORES = 8
NJC = N // 128           # 32 j-chunks

f32 = mybir.dt.float32
f32r = mybir.dt.float32r
bf16 = mybir.dt.bfloat16
i16 = mybir.dt.int16
FT = mybir.ActivationFunctionType
ALU = mybir.AluOpType
PSUM = bass.MemorySpace.PSUM

# Schraudolph bf16 exp on DVE: i16(round(x*A16 + B16)) bitcast bf16 ~ exp(x).
LN2 = 0.6931471805599453
A16 = 128.0 / LN2
B16 = 127.0 * 128.0 - 4.46   # magic-c correction balances error to ~±3.5%

N_DVE_PER_HALF = 14          # of 32 exp tiles per half on DVE (rest ACT)

# packed-constant layouts
PK_WQ, PK_WK, PK_SMALL = 0, 32, 64      # f32 pack: wq/wk rows 0:33, smalls
PKF = 68
PR_WVT, PR_ID32 = 0, 33                 # f32r pack
PRF = 65
PB_WCH, PB_WPOS = 0, 32                 # bf16 pack: stacked im2col weights
PBF = 64


def _dve_pattern(n_tiles, n_dve):
    out, acc = [], 0
    for _ in range(n_tiles):
        acc += n_dve
        if acc >= n_tiles:
            acc -= n_tiles
            out.append(True)
        else:
            out.append(False)
    return out


def _emit(tc, nc, t, out_d):
    with (
        tc.tile_pool(name="const", bufs=1) as cp,
        tc.tile_pool(name="work", bufs=1) as wp,
    ):
        # ---- SBUF tiles ----
        qTP_r = cp.tile([128, N], bf16)        # q^T replicated x4 (host-side)
        qTloc_r = cp.tile([C + 1, NL], f32r)   # local q_aug^T (f32 bits)
        qc2_b = cp.tile([128, NJC, 128], bf16)  # [data|ones] x2 AV weights
        qc_f = cp.tile([128, NJC, C + 1], f32)  # gram operand
        pk = cp.tile([128, PKF], f32)
        pkr = cp.tile([C, PRF], f32r)
        pkb = cp.tile([128, PBF], bf16)

        qTloc_f = qTloc_r[0:C, 0:NL].bitcast(f32)
        wq_f = pk[0:C + 1, PK_WQ:PK_WQ + C]
        wk_f = pk[0:C + 1, PK_WK:PK_WK + C]
        bch_v = pk[0:C, PK_SMALL:PK_SMALL + 1]
        bpos_v = pk[0:C, PK_SMALL + 1:PK_SMALL + 2]
        gamma_v = pk[0:C, PK_SMALL + 2:PK_SMALL + 3]
        beta128_v = pk[:, PK_SMALL + 3:PK_SMALL + 4]
        beta_v = pk[0:1, PK_SMALL + 3:PK_SMALL + 4]
        wvT_r = pkr[0:C, PR_WVT:PR_WVT + C + 1]
        id32_r = pkr[0:C, PR_ID32:PR_ID32 + C]
        wch_st = pkb[:, PB_WCH:PB_WCH + C]
        wpos_st = pkb[:, PB_WPOS:PB_WPOS + C]

        # ---- input DMAs spread across queues ----
        nc.sync.dma_start(qc_f[:], t["qcf"])       # gram operand first
        nc.gpsimd.dma_start(qTP_r[:], t["qT"])
        nc.scalar.dma_start(pk[:], t["pk"])
        nc.scalar.dma_start(pkr[:], t["pkr"])
        nc.scalar.dma_start(pkb[:], t["pkb"])
        nc.sync.dma_start(qTloc_r[:], t["qTloc"])
        nc.gpsimd.dma_start(qc2_b[:, :, 0:C], t["qc2d"])
        # trigger the ACT exp table load immediately (~1.3us)
        warm = wp.tile([1, 1], f32)
        nc.scalar.activation(warm[:], beta_v, FT.Exp)
        # qc2 = [data | ones] replicated onto both 64-column halves
        nc.vector.memset(qc2_b[:, :, C:2 * C], 1.0)
        nc.vector.tensor_copy(qc2_b[:, :, 2 * C:4 * C], qc2_b[:, :, 0:2 * C])

        relu_pos = wp.tile([C, NL], f32)
        relu_ch = wp.tile([C, NL], f32)
        sumT = wp.tile([C, NL], f32)
        paF_r = wp.tile([128, NL + 4], bf16)
        caF_r = wp.tile([128, NL + 4], bf16)
        tmp_ca = wp.tile([C, NL], f32)
        out_v = out_d.rearrange("(g kk r) f -> g r kk f", kk=16, r=C)

        # zero the conv-window pads: cols NL..NL+4 (block tail) and the
        # half-A/half-B seam cols 1024..1028 of caF (half A's shifted
        # replicas read 3 cols into not-yet-written half-B territory).
        nc.vector.memset(paF_r[:, NL:NL + 4], 0.0)
        nc.vector.memset(caF_r[:, NL:NL + 4], 0.0)
        nc.vector.memset(caF_r[:, NL // 2:NL // 2 + 4], 0.0)

        with tc.tile_pool(name="psAV", bufs=1, space=PSUM) as psAV:
            av0 = psAV.tile([128, 512], f32, tag="avA")  # half A: slices 0,1
            av1 = psAV.tile([128, 512], f32, tag="avB")  # half B: slices 2,3
            # gram quadrants: col-group c accumulates jc = c (mod 4); the four
            # [C, C] partial sums stack on partition groups of one psum bank.
            g_ps = psAV.tile([128, C], f32, tag="g")

            def emit_gram(j0, j1):
                for jc in range(j0, j1):
                    cq = jc & 3
                    nc.tensor.matmul(
                        g_ps[32 * cq:32 * cq + C, :],
                        qc_f[:, jc, 0:C], qc_f[:, jc, 0:C],
                        start=(jc < 4), stop=(jc >= NJC - 4),
                        tile_position=(0, 32 * cq), skip_group_check=True,
                    )

            def emit_small1():
                """G -> attn2 -> wpa (PE + tiny DVE/ACT ops)."""
                # G core: sum the 4 quadrant partials; aug row/col from a DVE
                # reduction of q^T (sum over all j); corner = N.
                g_sb = wp.tile([C + 1, C + 1], f32)
                gq1 = wp.tile([C, C], f32)
                gq2 = wp.tile([C, C], f32)
                gq3 = wp.tile([C, C], f32)
                nc.vector.tensor_copy(gq1[:], g_ps[C:2 * C, :])
                nc.vector.tensor_copy(gq2[:], g_ps[2 * C:3 * C, :])
                nc.vector.tensor_copy(gq3[:], g_ps[3 * C:4 * C, :])
                nc.vector.tensor_add(gq1[:], g_ps[0:C, :], gq1[:])
                nc.vector.tensor_add(gq2[:], gq2[:], gq3[:])
                nc.vector.tensor_add(g_sb[0:C, 0:C], gq1[:], gq2[:])
                csum = wp.tile([C, C], f32)
                nc.vector.memset(csum[:], 0.0)
                nc.vector.reduce_sum(csum[:, 0:1], qTP_r[0:C, :],
                                     axis=mybir.AxisListType.X)
                nc.vector.tensor_copy(g_sb[0:C, C:C + 1], csum[:, 0:1])
                csumT = wp.tile([C, C], f32)
                nc.vector.transpose(csumT[:], csum[:])
                nc.vector.tensor_copy(g_sb[C:C + 1, 0:C], csumT[0:1, :])
                nc.vector.memset(g_sb[C:C + 1, C:C + 1], float(N))
                t1_ps = psAV.tile([C + 1, C], f32, tag="g")
                nc.tensor.matmul(t1_ps[:], g_sb[:], wk_f, start=True, stop=True)
                t1_sb = wp.tile([C + 1, C], f32)
                nc.vector.tensor_copy(t1_sb[:], t1_ps[:])
                e2_ps = psAV.tile([C, C], f32, tag="g")
                nc.tensor.matmul(e2_ps[:], wq_f, t1_sb[:], start=True, stop=True)
                mx = wp.tile([C, 1], f32)
                nc.vector.reduce_max(mx[:], e2_ps[:], axis=mybir.AxisListType.X)
                nmx = wp.tile([C, 1], f32)
                nc.vector.tensor_scalar_mul(nmx[:], mx[:], -1.0)
                a_sb = wp.tile([C, C], f32)
                nc.scalar.activation(a_sb[:], e2_ps[:], FT.Exp, bias=nmx[:])
                sm = wp.tile([C, 1], f32)
                nc.vector.reduce_sum(sm[:], a_sb[:], axis=mybir.AxisListType.X)
                rc = wp.tile([C, 1], f32)
                nc.vector.reciprocal(rc[:], sm[:])
                a_n = wp.tile([C, C], f32r)
                nc.vector.tensor_scalar_mul(a_n[:], a_sb[:], rc[:])
                at_ps = psAV.tile([C, C], f32, tag="g")
                nc.tensor.matmul(at_ps[:], a_n[:], id32_r, start=True, stop=True)
                at_r = wp.tile([C, C], f32r)
                nc.vector.tensor_copy(at_r[:], at_ps[:])
                wpa_ps = psAV.tile([C + 1, C], f32, tag="g")
                nc.tensor.matmul(wpa_ps[:], wvT_r, at_r[:], start=True, stop=True)
                wpa_r = wp.tile([C + 1, C], f32r)
                nc.vector.tensor_copy(wpa_r[:], wpa_ps[:])
                return wpa_r

            def emit_small2(wpa_r):
                """pa branch + shifted replicas of paF."""
                for g in range(4):
                    pa_ps = psAV.tile([C, 512], f32, tag="g")
                    nc.tensor.matmul(
                        pa_ps[:], wpa_r[:], qTloc_r[:, g * 512:(g + 1) * 512],
                        start=True, stop=True,
                    )
                    nc.vector.scalar_tensor_tensor(
                        paF_r[0:C, g * 512:(g + 1) * 512], pa_ps[:], gamma_v,
                        qTloc_f[:, g * 512:(g + 1) * 512],
                        op0=ALU.mult, op1=ALU.add,
                    )
                for k in (1, 2, 3):
                    eng = (nc.gpsimd, nc.sync, nc.scalar)[k - 1]
                    eng.dma_start(
                        paF_r[32 * k:32 * k + C, 0:NL],
                        paF_r[0:C, k:k + NL],
                    )

            # ================= big branch: two half-phases =================
            s_roll = [0]   # rolling PE row-group for S matmuls

            def emit_av(av_t, jc, pt_ap):
                for p in range(2):
                    nc.tensor.matmul(
                        av_t[64 * p:64 * p + 64, :],
                        qc2_b[:, jc, 64 * p:64 * p + 64],
                        pt_ap[:, 512 * p:512 * p + 512],
                        start=(jc == 0), stop=(jc == NJC - 1),
                        tile_position=(0, 64 * p), skip_group_check=True,
                    )

            def emit_half(h, av_t, psS, ptp, dve_tiles, hooks):
                """S + exp + AV for slices (2h, 2h+1). One tile per jc; AV
                for jc is emitted after tile jc+1's exp (PE slack). hooks
                fire after the given tile index."""
                hooks = dict(hooks)
                pt_tiles = {}

                def s_tile(jc):
                    s_ps = psS.tile([128, 1024], f32, tag="s")
                    for s in range(2):
                        rp = s_roll[0] & 3
                        s_roll[0] += 1
                        cg = (2 * h + s) * 512
                        nc.tensor.matmul(
                            s_ps[:, s * 512:(s + 1) * 512],
                            qTP_r[32 * rp:32 * rp + C, jc * 128:(jc + 1) * 128],
                            qTP_r[32 * rp:32 * rp + C, cg:cg + 512],
                            start=True, stop=True,
                            tile_position=(32 * rp, 0), skip_group_check=True,
                        )
                    return s_ps

                def exp_tile(jc, s_ps):
                    if dve_tiles[jc]:
                        pti = ptp.tile([128, 1024], i16, tag="pt")
                        nc.vector.tensor_scalar(
                            pti[:], s_ps[:], A16, B16, op0=ALU.mult, op1=ALU.add,
                        )
                        pt_tiles[jc] = pti[:].bitcast(bf16)
                    else:
                        ptt = ptp.tile([128, 1024], bf16, tag="pt")
                        nc.scalar.activation(ptt[:], s_ps[:], FT.Exp)
                        pt_tiles[jc] = ptt[:]

                # 2-tile batches: [S S S S | exp exp | AV AV AV AV] -- four S
                # matmuls burst through all 4 PE row-groups, AV lags 2 tiles.
                for t0 in range(0, NJC, 2):
                    sp0 = s_tile(t0)
                    sp1 = s_tile(t0 + 1)
                    exp_tile(t0, sp0)
                    exp_tile(t0 + 1, sp1)
                    if t0 >= 2:
                        emit_av(av_t, t0 - 2, pt_tiles.pop(t0 - 2))
                        emit_av(av_t, t0 - 1, pt_tiles.pop(t0 - 1))
                    if t0 in hooks:
                        hooks.pop(t0)()
                    if t0 + 1 in hooks:
                        hooks.pop(t0 + 1)()
                emit_av(av_t, NJC - 2, pt_tiles.pop(NJC - 2))
                emit_av(av_t, NJC - 1, pt_tiles.pop(NJC - 1))
                for k in sorted(hooks):
                    hooks.pop(k)()

            def emit_norm(h, av_t, add_eng):
                """caF[0:C, h*1024:+1024] = beta*ca/denom + q_loc, then the
                shifted replicas onto row-groups 1..3."""
                c0 = h * 1024
                shf = wp.tile([128, 512], f32, tag=f"shf{h}")
                nc.vector.stream_shuffle(shf[:], av_t[:], [0] * 32)
                rec = wp.tile([128, 512], f32, tag=f"rec{h}")
                nc.vector.reciprocal_approx_fast(rec[:], shf[:])
                recB = wp.tile([128, 512], f32, tag=f"recB{h}")
                nc.vector.tensor_scalar_mul(recB[:], rec[:], beta128_v)
                for p in range(2):
                    nc.vector.tensor_mul(
                        tmp_ca[:, c0 + p * 512:c0 + (p + 1) * 512],
                        av_t[64 * p:64 * p + C, :],
                        recB[64 * p + C:64 * p + 2 * C, :],
                    )
                    add_eng.tensor_add(
                        caF_r[0:C, c0 + p * 512:c0 + (p + 1) * 512],
                        tmp_ca[:, c0 + p * 512:c0 + (p + 1) * 512],
                        qTloc_f[:, c0 + p * 512:c0 + (p + 1) * 512],
                    )
                for k in (1, 2, 3):
                    eng = (nc.gpsimd, nc.sync, nc.gpsimd)[k - 1]
                    eng.dma_start(
                        caF_r[32 * k:32 * k + C, c0:c0 + 1024],
                        caF_r[0:C, c0 + k:c0 + k + 1024],
                    )

            def conv_chunk(pool, tag, wst, x2t, bias_v, relu_out, cu):
                """one [C, 512] conv chunk: single K=128 im2col matmul."""
                cv = pool.tile([C, 512], f32, tag=tag)
                nc.tensor.matmul(
                    cv[:], wst, x2t[:, cu * 512:cu * 512 + 512],
                    start=True, stop=True,
                )
                nc.scalar.activation(
                    relu_out[:, cu * 512:(cu + 1) * 512], cv[:],
                    FT.Relu, bias=bias_v,
                )

            def emit_convs(h, pool, tags, branches=("ch", "pos")):
                i = 0
                for cu in (2 * h, 2 * h + 1):
                    if "ch" in branches:
                        conv_chunk(pool, tags[i % len(tags)], wch_st, caF_r,
                                   bch_v, relu_ch, cu)
                        i += 1
                    if "pos" in branches:
                        conv_chunk(pool, tags[i % len(tags)], wpos_st, paF_r,
                                   bpos_v, relu_pos, cu)
                        i += 1

            def emit_outs(h, add_eng):
                for cu in (2 * h, 2 * h + 1):
                    add_eng.tensor_add(
                        sumT[:, cu * 512:(cu + 1) * 512],
                        relu_ch[:, cu * 512:(cu + 1) * 512],
                        relu_pos[:, cu * 512:(cu + 1) * 512],
                    )
                    tb = wp.tile([C, 512], f32, tag=f"ob{cu & 1}")
                    nc.vector.transpose(tb[:], sumT[:, cu * 512:(cu + 1) * 512])
                    eng = nc.sync if cu % 2 == 0 else nc.gpsimd
                    eng.dma_start(
                        out_v[cu],
                        tb[:].rearrange("r (kk f) -> r kk f", kk=16),
                    )

            patA = _dve_pattern(NJC, N_DVE_PER_HALF)
            patB = _dve_pattern(NJC, N_DVE_PER_HALF)
            wpa_box = {}
            with (
                tc.tile_pool(name="psS", bufs=2, space=PSUM) as psS,
                tc.tile_pool(name="ptp", bufs=8) as ptp,
            ):
                # gram: first chunks fill the head DMA wait; the rest are PE
                # bubble filler between half-A tiles.
                emit_gram(0, 8)
                hooksA = {
                    1: lambda: emit_gram(8, 14),
                    3: lambda: emit_gram(14, 20),
                    5: lambda: emit_gram(20, 26),
                    7: lambda: emit_gram(26, 32),
                    9: lambda: wpa_box.__setitem__("w", emit_small1()),
                    11: lambda: emit_small2(wpa_box["w"]),
                }
                emit_half(0, av0, psS, ptp, patA, hooksA)
                hooksB = {
                    0: lambda: emit_norm(0, av0, nc.gpsimd),
                    6: lambda: emit_convs(0, psAV, ["g", "g2"]),
                    14: lambda: emit_outs(0, nc.gpsimd),
                    22: lambda: emit_convs(1, psAV, ["g", "g2"],
                                           branches=("pos",)),
                }
                emit_half(1, av1, psS, ptp, patB, hooksB)
                emit_norm(1, av1, nc.vector)
            # psS closed: banks free for a pipelined final conv
            with tc.tile_pool(name="psC2", bufs=2, space=PSUM) as psC2:
                emit_convs(1, psC2, ["cv"], branches=("ch",))
                emit_outs(1, nc.vector)


def _build():
    nc = bacc.Bacc("TRN2", target_bir_lowering=False, debug=False)
    t = {}

    def din(name, shape, dt):
        t[name] = nc.dram_tensor(name, shape, dt, kind="ExternalInput").ap()

    din("qT", [128, N], bf16)
    din("qTloc", [C + 1, NL], f32r)
    din("qc2d", [128, NJC, C], bf16)
    din("qcf", [128, NJC, C + 1], f32)
    din("pk", [128, PKF], f32)
    din("pkr", [C, PRF], f32r)
    din("pkb", [128, PBF], bf16)
    out_d = nc.dram_tensor("out", [NL, C], f32, kind="ExternalOutput").ap()

    with tile.TileContext(nc) as tc:
        _emit(tc, nc, t, out_d)
    nc.compile()
    return nc


_NC = None


def _get_nc():
    global _NC
    if _NC is None:
        _NC = _build()
    return _NC


def _prepare_in_maps(inputs):
    x = np.asarray(inputs["inputs"], np.float32)
    beta = np.asarray(inputs["beta"], np.float32)
    gamma = np.asarray(inputs["gamma"], np.float32)
    wq_aug = np.concatenate(
        [np.asarray(inputs["wq"], np.float32), np.asarray(inputs["bq"], np.float32)[None, :]], 0
    )
    wk_aug = np.concatenate(
        [np.asarray(inputs["wk"], np.float32), np.asarray(inputs["bk"], np.float32)[None, :]], 0
    )
    wv_aug = np.concatenate(
        [np.asarray(inputs["wv"], np.float32), np.asarray(inputs["bv"], np.float32)[None, :]], 0
    )
    pk = np.zeros((128, PKF), np.float32)
    pk[0:C + 1, PK_WQ:PK_WQ + C] = wq_aug
    pk[0:C + 1, PK_WK:PK_WK + C] = wk_aug
    pk[0:C, PK_SMALL] = np.asarray(inputs["b_ch"], np.float32)
    pk[0:C, PK_SMALL + 1] = np.asarray(inputs["b_pos"], np.float32)
    pk[0:C, PK_SMALL + 2] = gamma[0]
    pk[0:128, PK_SMALL + 3] = beta[0]
    pkr = np.zeros((C, PRF), np.float32)
    pkr[0:C, PR_WVT:PR_WVT + C + 1] = wv_aug.T
    pkr[0:C, PR_ID32:PR_ID32 + C] = np.eye(C, dtype=np.float32)
    pkb = np.zeros((128, PBF), np.float32)
    pkb[:, PB_WCH:PB_WCH + C] = np.asarray(inputs["w_ch"], np.float32).reshape(4 * C, C)
    pkb[:, PB_WPOS:PB_WPOS + C] = np.asarray(inputs["w_pos"], np.float32).reshape(4 * C, C)
    pkb = pkb.astype(ml_dtypes.bfloat16)

    in_maps = []
    for core in range(NCORES):
        b, s = core // 2, core % 2
        qs = x[b].reshape(N, C)
        # local-half-first column permutation: S_T rhs slices [0, NL) are the
        # core's own rows; softmax sums over all j are order-invariant.
        q = np.concatenate([qs[s * NL:(s + 1) * NL], qs[(1 - s) * NL:(2 - s) * NL]])
        q_aug = np.concatenate([q, np.ones((N, 1), np.float32)], 1)
        qloc_aug = q_aug[:NL]
        qc = np.ascontiguousarray(q_aug.reshape(NJC, 128, C + 1).transpose(1, 0, 2))
        qT_b = np.ascontiguousarray(q.T).astype(ml_dtypes.bfloat16)
        m = {
            "qT": np.ascontiguousarray(np.tile(qT_b, (4, 1))),
            "qTloc": np.ascontiguousarray(qloc_aug.T),
            "qc2d": np.ascontiguousarray(qc[:, :, :C]).astype(ml_dtypes.bfloat16),
            "qcf": qc,
            "pk": pk,
            "pkr": pkr,
            "pkb": pkb,
        }
        in_maps.append(m)
    return in_maps


def _run(inputs, trace=False):
    nc = _get_nc()
    in_maps = _prepare_in_maps(inputs)
    res = bass_utils.run_bass_kernel_spmd(
        nc, in_maps, core_ids=list(range(NCORES)), trace=trace
    )
    out = np.empty((B, H, W, DO, C), np.float32)
    for core in range(NCORES):
        b, s = core // 2, core % 2
        full = res.results[core]["out"].reshape(8, W, D, C)
        out[b, s * 8:(s + 1) * 8] = full[:, :, :DO, :]
    return out, res


def kernel(**inputs):
    out, _ = _run(inputs, trace=False)
    return out


# revision 22
# speedup vs baseline: 1.4060x; 1.0082x over previous
"""Trainium2 Bass kernel for nn_Attention_Embedding (spatial NxN attention +
channel CxC attention + conv3d(1,1,4) embedding head).

Sharding: 8 cores = 4 samples x 2 halves (split on H). Each core holds its
sample's full q (softmax rows are complete) and produces its own slice of the
final output; no cross-core communication.

v3 (from the v2 trace: PE-bound at the HAM-throttled 1.2GHz clock, ~300ns
fixed cost per matmul, S/AV subarray-conflict serialization):
  - exp split across ACT (exact, bf16 out) and DVE (Schraudolph bit-trick:
    i16(round(x*128/ln2 + B)) bitcast to bf16, ~±3.5% per element).
  - S tiles are [128,1024] = one j-chunk x both slices of the half; AV for
    jc is emitted one tile late so the PE never stalls waiting for exp.
  - conv3d via true im2col: the 4 row-group replicas of caF/paF are SHIFTED
    copies (replica k holds x[c, n+k]) so each conv chunk is ONE K=128
    matmul instead of 4 accumulating K=32 matmuls.
  - gram (small-branch) matmuls interleaved into half A as PE bubble filler.
  - two half-phases over i so half A's tail overlaps half B's S/exp/AV.
  - denominator broadcast via DVE stream_shuffle; tail elementwise on GPSIMD.
"""

import os
import sys

for _p in ("/opt/trn_rl_repo", "/root/.axon_site/_ro/trn_rl_repo"):
    if os.path.isdir(_p) and _p not in sys.path:
        sys.path.insert(0, _p)
        break

import ml_dtypes
import numpy as np

import concourse.bacc as bacc
import concourse.bass as bass
import concourse.mybir as mybir
import concourse.tile as tile
from concourse import bass_utils

B, H, W, D, C = 4, 16, 16, 16, 32
N = H * W * D            # 4096
NL = N // 2              # 2048 rows per core
DO = D - 3               # 13 conv output positions
NCORES = 8
NJC = N // 128           # 32 j-chunks

f32 = mybir.dt.float32
f32r = mybir.dt.float32r
bf16 = mybir.dt.bfloat16
i16 = mybir.dt.int16
FT = mybir.ActivationFunctionType
ALU = mybir.AluOpType
PSUM = bass.MemorySpace.PSUM

# Schraudolph bf16 exp on DVE: i16(round(x*A16 + B16)) bitcast bf16 ~ exp(x).
LN2 = 0.6931471805599453
A16 = 128.0 / LN2
B16 = 127.0 * 128.0 - 4.46   # magic-c correction balances error to ~±3.5%

N_DVE_PER_HALF = 14          # of 32 exp tiles per half on DVE (rest ACT)

# packed-constant layouts
PK_WQ, PK_WK, PK_SMALL = 0, 32, 64      # f32 pack: wq/wk rows 0:33, smalls
PKF = 68
PR_WVT, PR_ID32 = 0, 33                 # f32r pack
PRF = 65
# bf16 pack: stacked im2col weights [128,32] + flat k-major forms [32,128]
PB_WCH, PB_WPOS, PB_WCH4, PB_WPOS4 = 0, 32, 64, 192
PBF = 320


def _dve_pattern(n_tiles, n_dve):
    out, acc = [], 0
    for _ in range(n_tiles):
        acc += n_dve
        if acc >= n_tiles:
            acc -= n_tiles
            out.append(True)
        else:
            out.append(False)
    return out


def _emit(tc, nc, t, out_d):
    with (
        tc.tile_pool(name="const", bufs=1) as cp,
        tc.tile_pool(name="work", bufs=1) as wp,
    ):
        # ---- SBUF tiles ----
        qTP_r = cp.tile([128, N], bf16)        # q^T replicated x4 (host-side)
        qTloc_r = cp.tile([C + 1, NL], f32r)   # local q_aug^T (f32 bits)
        qc2_b = cp.tile([128, NJC, 128], bf16)  # [data|ones] x2 AV weights
        qc_f = cp.tile([128, NJC, C + 1], f32)  # gram operand
        pk = cp.tile([128, PKF], f32)
        pkr = cp.tile([C, PRF], f32r)
        pkb = cp.tile([128, PBF], bf16)

        qTloc_f = qTloc_r[0:C, 0:NL].bitcast(f32)
        wq_f = pk[0:C + 1, PK_WQ:PK_WQ + C]
        wk_f = pk[0:C + 1, PK_WK:PK_WK + C]
        bch_v = pk[0:C, PK_SMALL:PK_SMALL + 1]
        bpos_v = pk[0:C, PK_SMALL + 1:PK_SMALL + 2]
        gamma_v = pk[0:C, PK_SMALL + 2:PK_SMALL + 3]
        beta128_v = pk[:, PK_SMALL + 3:PK_SMALL + 4]
        beta_v = pk[0:1, PK_SMALL + 3:PK_SMALL + 4]
        wvT_r = pkr[0:C, PR_WVT:PR_WVT + C + 1]
        id32_r = pkr[0:C, PR_ID32:PR_ID32 + C]
        wch_st = pkb[:, PB_WCH:PB_WCH + C]
        wpos_st = pkb[:, PB_WPOS:PB_WPOS + C]
        wch4_flat = pkb[0:C, PB_WCH4:PB_WCH4 + 4 * C]
        wpos4_flat = pkb[0:C, PB_WPOS4:PB_WPOS4 + 4 * C]

        # ---- input DMAs spread across queues ----
        nc.sync.dma_start(qc_f[:, 0:8, :], t["qcf"][:, 0:8, :])
        nc.gpsimd.dma_start(qTP_r[0:C, :], t["qT"])
        nc.scalar.dma_start(pk[:], t["pk"])
        nc.scalar.dma_start(pkr[:], t["pkr"])
        nc.scalar.dma_start(pkb[:], t["pkb"])
        nc.sync.dma_start(qc_f[:, 8:NJC, :], t["qcf"][:, 8:NJC, :])
        nc.sync.dma_start(qTloc_r[:], t["qTloc"])
        nc.gpsimd.dma_start(qc2_b[:, :, 0:C], t["qc2d"])
        # trigger the ACT exp table load immediately (~1.3us)
        warm = wp.tile([1, 1], f32)
        nc.scalar.activation(warm[:], beta_v, FT.Exp)
        # replicate q^T onto row-groups 1..3 for the rolling S matmuls
        nc.vector.tensor_copy(qTP_r[C:2 * C, :], qTP_r[0:C, :])
        nc.vector.tensor_copy(qTP_r[2 * C:4 * C, :], qTP_r[0:2 * C, :])
        # qc2 = [data | ones] replicated onto both 64-column halves
        nc.vector.memset(qc2_b[:, :, C:2 * C], 1.0)
        nc.vector.tensor_copy(qc2_b[:, :, 2 * C:4 * C], qc2_b[:, :, 0:2 * C])

        relu_pos = wp.tile([C, NL], f32)
        relu_ch = wp.tile([C, NL], f32)
        sumT = wp.tile([C, NL], f32)
        paF_r = wp.tile([128, NL + 4], bf16)
        caF_r = wp.tile([128, NL + 4], bf16)
        tmp_ca = wp.tile([C, NL], f32)
        out_v = out_d.rearrange("(g kk r) f -> g r kk f", kk=16, r=C)

        # zero the conv-window pads: cols NL..NL+4 (block tail) and the
        # half-A/half-B seam cols 1024..1028 of caF (half A's shifted
        # replicas read 3 cols into not-yet-written half-B territory).
        nc.vector.memset(paF_r[:, NL:NL + 4], 0.0)
        nc.vector.memset(caF_r[:, NL:NL + 4], 0.0)
        nc.vector.memset(caF_r[:, NL // 2:NL // 2 + 4], 0.0)

        with tc.tile_pool(name="psAV", bufs=1, space=PSUM) as psAV:
            av0 = psAV.tile([128, 512], f32, tag="avA")  # half A: slices 0,1
            av1 = psAV.tile([128, 512], f32, tag="avB")  # half B: slices 2,3
            # gram quadrants: col-group c accumulates jc = c (mod 4); the four
            # [C, C] partial sums stack on partition groups of one psum bank.
            g_ps = psAV.tile([128, C], f32, tag="g")

            def emit_gram(j0, j1):
                for jc in range(j0, j1):
                    cq = jc & 3
                    nc.tensor.matmul(
                        g_ps[32 * cq:32 * cq + C, :],
                        qc_f[:, jc, 0:C], qc_f[:, jc, 0:C],
                        start=(jc < 4), stop=(jc >= NJC - 4),
                        tile_position=(0, 32 * cq), skip_group_check=True,
                    )

            def emit_small1():
                """G -> attn2 -> wpa (PE + tiny DVE/ACT ops)."""
                # G core: sum the 4 quadrant partials; aug row/col from a DVE
                # reduction of q^T (sum over all j); corner = N.
                g_sb = wp.tile([C + 1, C + 1], f32)
                gq1 = wp.tile([C, C], f32)
                gq2 = wp.tile([C, C], f32)
                gq3 = wp.tile([C, C], f32)
                nc.vector.tensor_copy(gq1[:], g_ps[C:2 * C, :])
                nc.vector.tensor_copy(gq2[:], g_ps[2 * C:3 * C, :])
                nc.vector.tensor_copy(gq3[:], g_ps[3 * C:4 * C, :])
                nc.vector.tensor_add(gq1[:], g_ps[0:C, :], gq1[:])
                nc.vector.tensor_add(gq2[:], gq2[:], gq3[:])
                nc.vector.tensor_add(g_sb[0:C, 0:C], gq1[:], gq2[:])
                csum = wp.tile([C, C], f32)
                nc.vector.memset(csum[:], 0.0)
                nc.vector.reduce_sum(csum[:, 0:1], qTP_r[0:C, :],
                                     axis=mybir.AxisListType.X)
                nc.vector.tensor_copy(g_sb[0:C, C:C + 1], csum[:, 0:1])
                csumT = wp.tile([C, C], f32)
                nc.vector.transpose(csumT[:], csum[:])
                nc.vector.tensor_copy(g_sb[C:C + 1, 0:C], csumT[0:1, :])
                nc.vector.memset(g_sb[C:C + 1, C:C + 1], float(N))
                t1_ps = psAV.tile([C + 1, C], f32, tag="g")
                nc.tensor.matmul(t1_ps[:], g_sb[:], wk_f, start=True, stop=True)
                t1_sb = wp.tile([C + 1, C], f32)
                nc.vector.tensor_copy(t1_sb[:], t1_ps[:])
                e2_ps = psAV.tile([C, C], f32, tag="g")
                nc.tensor.matmul(e2_ps[:], wq_f, t1_sb[:], start=True, stop=True)
                mx = wp.tile([C, 1], f32)
                nc.vector.reduce_max(mx[:], e2_ps[:], axis=mybir.AxisListType.X)
                nmx = wp.tile([C, 1], f32)
                nc.vector.tensor_scalar_mul(nmx[:], mx[:], -1.0)
                a_sb = wp.tile([C, C], f32)
                nc.scalar.activation(a_sb[:], e2_ps[:], FT.Exp, bias=nmx[:])
                sm = wp.tile([C, 1], f32)
                nc.vector.reduce_sum(sm[:], a_sb[:], axis=mybir.AxisListType.X)
                rc = wp.tile([C, 1], f32)
                nc.vector.reciprocal(rc[:], sm[:])
                a_n = wp.tile([C, C], f32r)
                nc.vector.tensor_scalar_mul(a_n[:], a_sb[:], rc[:])
                at_ps = psAV.tile([C, C], f32, tag="g")
                nc.tensor.matmul(at_ps[:], a_n[:], id32_r, start=True, stop=True)
                at_r = wp.tile([C, C], f32r)
                nc.vector.tensor_copy(at_r[:], at_ps[:])
                wpa_ps = psAV.tile([C + 1, C], f32, tag="g")
                nc.tensor.matmul(wpa_ps[:], wvT_r, at_r[:], start=True, stop=True)
                wpa_r = wp.tile([C + 1, C], f32r)
                nc.vector.tensor_copy(wpa_r[:], wpa_ps[:])
                return wpa_r

            def emit_small2(wpa_r):
                """pa branch + shifted replicas of paF."""
                for g in range(4):
                    pa_ps = psAV.tile([C, 512], f32, tag="g")
                    nc.tensor.matmul(
                        pa_ps[:], wpa_r[:], qTloc_r[:, g * 512:(g + 1) * 512],
                        start=True, stop=True,
                    )
                    nc.vector.scalar_tensor_tensor(
                        paF_r[0:C, g * 512:(g + 1) * 512], pa_ps[:], gamma_v,
                        qTloc_f[:, g * 512:(g + 1) * 512],
                        op0=ALU.mult, op1=ALU.add,
                    )
                for k in (1, 2, 3):
                    eng = (nc.gpsimd, nc.sync, nc.scalar)[k - 1]
                    eng.dma_start(
                        paF_r[32 * k:32 * k + C, 0:NL],
                        paF_r[0:C, k:k + NL],
                    )

            # ================= big branch: two half-phases =================
            s_roll = [0]   # rolling PE row-group for S matmuls

            def emit_av(av_t, jc, pt_ap):
                for p in range(2):
                    nc.tensor.matmul(
                        av_t[64 * p:64 * p + 64, :],
                        qc2_b[:, jc, 64 * p:64 * p + 64],
                        pt_ap[:, 512 * p:512 * p + 512],
                        start=(jc == 0), stop=(jc == NJC - 1),
                        tile_position=(0, 64 * p), skip_group_check=True,
                    )

            def emit_half(h, av_t, psS, ptp_a, ptp_d, dve_tiles, hooks):
                """S + exp + AV for slices (2h, 2h+1). One tile per jc; AV
                for jc is emitted after tile jc+1's exp (PE slack). hooks
                fire after the given tile index."""
                hooks = dict(hooks)
                pt_tiles = {}

                def s_tile(jc):
                    s_ps = psS.tile([128, 1024], f32, tag="s")
                    for s in range(2):
                        rp = s_roll[0] & 3
                        s_roll[0] += 1
                        cg = (2 * h + s) * 512
                        nc.tensor.matmul(
                            s_ps[:, s * 512:(s + 1) * 512],
                            qTP_r[32 * rp:32 * rp + C, jc * 128:(jc + 1) * 128],
                            qTP_r[32 * rp:32 * rp + C, cg:cg + 512],
                            start=True, stop=True,
                            tile_position=(32 * rp, 0), skip_group_check=True,
                        )
                    return s_ps

                def exp_tile(jc, s_ps):
                    if dve_tiles[jc]:
                        pti = ptp_d.tile([128, 1024], i16, tag="ptd")
                        nc.vector.tensor_scalar(
                            pti[:], s_ps[:], A16, B16, op0=ALU.mult, op1=ALU.add,
                        )
                        pt_tiles[jc] = pti[:].bitcast(bf16)
                    else:
                        ptt = ptp_a.tile([128, 1024], bf16, tag="pta")
                        nc.scalar.activation(ptt[:], s_ps[:], FT.Exp)
                        pt_tiles[jc] = ptt[:]

                # 2-tile batches: [S S S S | exp exp | AV AV AV AV] -- four S
                # matmuls burst through all 4 PE row-groups, AV lags 2 tiles.
                for t0 in range(0, NJC, 2):
                    sp0 = s_tile(t0)
                    sp1 = s_tile(t0 + 1)
                    exp_tile(t0, sp0)
                    exp_tile(t0 + 1, sp1)
                    if t0 >= 2:
                        emit_av(av_t, t0 - 2, pt_tiles.pop(t0 - 2))
                        emit_av(av_t, t0 - 1, pt_tiles.pop(t0 - 1))
                    if t0 in hooks:
                        hooks.pop(t0)()
                    if t0 + 1 in hooks:
                        hooks.pop(t0 + 1)()
                emit_av(av_t, NJC - 2, pt_tiles.pop(NJC - 2))
                emit_av(av_t, NJC - 1, pt_tiles.pop(NJC - 1))
                for k in sorted(hooks):
                    hooks.pop(k)()

            def emit_norm(h, av_t, add_eng, replicas=True):
                """caF[0:C, h*1024:+1024] = beta*ca/denom + q_loc, then the
                shifted replicas onto row-groups 1..3."""
                c0 = h * 1024
                shf = wp.tile([128, 512], f32, tag=f"shf{h}")
                nc.vector.stream_shuffle(shf[:], av_t[:], [0] * 32)
                rec = wp.tile([128, 512], f32, tag=f"rec{h}")
                nc.vector.reciprocal_approx_fast(rec[:], shf[:])
                recB = wp.tile([128, 512], f32, tag=f"recB{h}")
                nc.vector.tensor_scalar_mul(recB[:], rec[:], beta128_v)
                for p in range(2):
                    nc.vector.tensor_mul(
                        tmp_ca[:, c0 + p * 512:c0 + (p + 1) * 512],
                        av_t[64 * p:64 * p + C, :],
                        recB[64 * p + C:64 * p + 2 * C, :],
                    )
                    add_eng.tensor_add(
                        caF_r[0:C, c0 + p * 512:c0 + (p + 1) * 512],
                        tmp_ca[:, c0 + p * 512:c0 + (p + 1) * 512],
                        qTloc_f[:, c0 + p * 512:c0 + (p + 1) * 512],
                    )
                if replicas:
                    for k in (1, 2, 3):
                        eng = (nc.gpsimd, nc.sync, nc.gpsimd)[k - 1]
                        eng.dma_start(
                            caF_r[32 * k:32 * k + C, c0:c0 + 1024],
                            caF_r[0:C, c0 + k:c0 + k + 1024],
                        )

            def conv_chunk(pool, tag, wst, x2t, bias_v, relu_out, cu,
                           kshift=None):
                """one [C, 512] conv chunk: a single K=128 im2col matmul on
                the shifted replicas, or (kshift) 4 accumulating K=32
                matmuls reading only row-group 0 (no replicas needed)."""
                cv = pool.tile([C, 512], f32, tag=tag)
                if kshift is None:
                    nc.tensor.matmul(
                        cv[:], wst, x2t[:, cu * 512:cu * 512 + 512],
                        start=True, stop=True,
                    )
                else:
                    for k in range(4):
                        nc.tensor.matmul(
                            cv[:],
                            kshift[:, k * C:(k + 1) * C],
                            x2t[0:C, cu * 512 + k:cu * 512 + k + 512],
                            start=(k == 0), stop=(k == 3),
                        )
                nc.scalar.activation(
                    relu_out[:, cu * 512:(cu + 1) * 512], cv[:],
                    FT.Relu, bias=bias_v,
                )

            def emit_convs(h, pool, tags, branches=("ch", "pos"), ks=False):
                i = 0
                for cu in (2 * h, 2 * h + 1):
                    if "ch" in branches:
                        conv_chunk(pool, tags[i % len(tags)], wch_st, caF_r,
                                   bch_v, relu_ch, cu,
                                   kshift=wch4_flat if ks else None)
                        i += 1
                    if "pos" in branches:
                        conv_chunk(pool, tags[i % len(tags)], wpos_st, paF_r,
                                   bpos_v, relu_pos, cu,
                                   kshift=wpos4_flat if ks else None)
                        i += 1

            def emit_outs(h, add_eng):
                for cu in (2 * h, 2 * h + 1):
                    add_eng.tensor_add(
                        sumT[:, cu * 512:(cu + 1) * 512],
                        relu_ch[:, cu * 512:(cu + 1) * 512],
                        relu_pos[:, cu * 512:(cu + 1) * 512],
                    )
                    tb = wp.tile([C, 512], f32, tag=f"ob{cu & 1}")
                    nc.vector.transpose(tb[:], sumT[:, cu * 512:(cu + 1) * 512])
                    eng = nc.sync if cu % 2 == 0 else nc.gpsimd
                    eng.dma_start(
                        out_v[cu],
                        tb[:].rearrange("r (kk f) -> r kk f", kk=16),
                    )

            patA = _dve_pattern(NJC, N_DVE_PER_HALF)
            patB = _dve_pattern(NJC, N_DVE_PER_HALF)
            wpa_box = {}
            with (
                tc.tile_pool(name="psS", bufs=2, space=PSUM) as psS,
                tc.tile_pool(name="ptpa", bufs=5) as ptp_a,
                tc.tile_pool(name="ptpd", bufs=5) as ptp_d,
            ):
                # gram: first chunks fill the head DMA wait; the rest are PE
                # bubble filler between half-A tiles.
                emit_gram(0, 8)
                hooksA = {
                    1: lambda: emit_gram(8, 14),
                    3: lambda: emit_gram(14, 20),
                    5: lambda: emit_gram(20, 26),
                    7: lambda: emit_gram(26, 32),
                    9: lambda: wpa_box.__setitem__("w", emit_small1()),
                    11: lambda: emit_small2(wpa_box["w"]),
                }
                emit_half(0, av0, psS, ptp_a, ptp_d, patA, hooksA)
                hooksB = {
                    0: lambda: emit_norm(0, av0, nc.gpsimd),
                    6: lambda: emit_convs(0, psAV, ["g", "g2"]),
                    14: lambda: emit_outs(0, nc.gpsimd),
                    22: lambda: emit_convs(1, psAV, ["g", "g2"],
                                           branches=("pos",)),
                }
                emit_half(1, av1, psS, ptp_a, ptp_d, patB, hooksB)
                emit_norm(1, av1, nc.vector, replicas=False)
            # psS closed: banks free for a pipelined final conv
            with tc.tile_pool(name="psC2", bufs=2, space=PSUM) as psC2:
                emit_convs(1, psC2, ["cv"], branches=("ch",), ks=True)
                emit_outs(1, nc.vector)


def _build():
    nc = bacc.Bacc("TRN2", target_bir_lowering=False, debug=False)
    t = {}

    def din(name, shape, dt):
        t[name] = nc.dram_tensor(name, shape, dt, kind="ExternalInput").ap()

    din("qT", [C, N], bf16)
    din("qTloc", [C + 1, NL], f32r)
    din("qc2d", [128, NJC, C], bf16)
    din("qcf", [128, NJC, C + 1], f32)
    din("pk", [128, PKF], f32)
    din("pkr", [C, PRF], f32r)
    din("pkb", [128, PBF], bf16)
    out_d = nc.dram_tensor("out", [NL, C], f32, kind="ExternalOutput").ap()

    with tile.TileContext(nc) as tc:
        _emit(tc, nc, t, out_d)
    nc.compile()
    return nc


_NC = None


def _get_nc():
    global _NC
    if _NC is None:
        _NC = _build()
    return _NC


def _prepare_in_maps(inputs):
    x = np.asarray(inputs["inputs"], np.float32)
    beta = np.asarray(inputs["beta"], np.float32)
    gamma = np.asarray(inputs["gamma"], np.float32)
    wq_aug = np.concatenate(
        [np.asarray(inputs["wq"], np.float32), np.asarray(inputs["bq"], np.float32)[None, :]], 0
    )
    wk_aug = np.concatenate(
        [np.asarray(inputs["wk"], np.float32), np.asarray(inputs["bk"], np.float32)[None, :]], 0
    )
    wv_aug = np.concatenate(
        [np.asarray(inputs["wv"], np.float32), np.asarray(inputs["bv"], np.float32)[None, :]], 0
    )
    pk = np.zeros((128, PKF), np.float32)
    pk[0:C + 1, PK_WQ:PK_WQ + C] = wq_aug
    pk[0:C + 1, PK_WK:PK_WK + C] = wk_aug
    pk[0:C, PK_SMALL] = np.asarray(inputs["b_ch"], np.float32)
    pk[0:C, PK_SMALL + 1] = np.asarray(inputs["b_pos"], np.float32)
    pk[0:C, PK_SMALL + 2] = gamma[0]
    pk[0:128, PK_SMALL + 3] = beta[0]
    pkr = np.zeros((C, PRF), np.float32)
    pkr[0:C, PR_WVT:PR_WVT + C + 1] = wv_aug.T
    pkr[0:C, PR_ID32:PR_ID32 + C] = np.eye(C, dtype=np.float32)
    pkb = np.zeros((128, PBF), np.float32)
    wch3 = np.asarray(inputs["w_ch"], np.float32).reshape(4, C, C)
    wpos3 = np.asarray(inputs["w_pos"], np.float32).reshape(4, C, C)
    pkb[:, PB_WCH:PB_WCH + C] = wch3.reshape(4 * C, C)
    pkb[:, PB_WPOS:PB_WPOS + C] = wpos3.reshape(4 * C, C)
    pkb[0:C, PB_WCH4:PB_WCH4 + 4 * C] = wch3.transpose(1, 0, 2).reshape(C, 4 * C)
    pkb[0:C, PB_WPOS4:PB_WPOS4 + 4 * C] = wpos3.transpose(1, 0, 2).reshape(C, 4 * C)
    pkb = pkb.astype(ml_dtypes.bfloat16)

    in_maps = []
    for core in range(NCORES):
        b, s = core // 2, core % 2
        qs = x[b].reshape(N, C)
        # local-half-first column permutation: S_T rhs slices [0, NL) are the
        # core's own rows; softmax sums over all j are order-invariant.
        q = np.concatenate([qs[s * NL:(s + 1) * NL], qs[(1 - s) * NL:(2 - s) * NL]])
        q_aug = np.concatenate([q, np.ones((N, 1), np.float32)], 1)
        qloc_aug = q_aug[:NL]
        qc = np.ascontiguousarray(q_aug.reshape(NJC, 128, C + 1).transpose(1, 0, 2))
        qT_b = np.ascontiguousarray(q.T).astype(ml_dtypes.bfloat16)
        m = {
            "qT": qT_b,
            "qTloc": np.ascontiguousarray(qloc_aug.T),
            "qc2d": np.ascontiguousarray(qc[:, :, :C]).astype(ml_dtypes.bfloat16),
            "qcf": qc,
            "pk": pk,
            "pkr": pkr,
            "pkb": pkb,
        }
        in_maps.append(m)
    return in_maps


def _run(inputs, trace=False):
    nc = _get_nc()
    in_maps = _prepare_in_maps(inputs)
    res = bass_utils.run_bass_kernel_spmd(
        nc, in_maps, core_ids=list(range(NCORES)), trace=trace
    )
    out = np.empty((B, H, W, DO, C), np.float32)
    for core in range(NCORES):
        b, s = core // 2, core % 2
        full = res.results[core]["out"].reshape(8, W, D, C)
        out[b, s * 8:(s + 1) * 8] = full[:, :, :DO, :]
    return out, res


def kernel(**inputs):
    out, _ = _run(inputs, trace=False)
    return out


# revision 23
# speedup vs baseline: 1.5951x; 1.1345x over previous
"""Trainium2 Bass kernel for nn_Attention_Embedding (spatial NxN attention +
channel CxC attention + conv3d(1,1,4) embedding head).

Sharding: 8 cores = 4 samples x 2 halves (split on H). Each core holds its
sample's full q (softmax rows are complete) and produces its own slice of the
final output; no cross-core communication.

v3 (from the v2 trace: PE-bound at the HAM-throttled 1.2GHz clock, ~300ns
fixed cost per matmul, S/AV subarray-conflict serialization):
  - exp split across ACT (exact, bf16 out) and DVE (Schraudolph bit-trick:
    i16(round(x*128/ln2 + B)) bitcast to bf16, ~±3.5% per element).
  - S tiles are [128,1024] = one j-chunk x both slices of the half; AV for
    jc is emitted one tile late so the PE never stalls waiting for exp.
  - conv3d via true im2col: the 4 row-group replicas of caF/paF are SHIFTED
    copies (replica k holds x[c, n+k]) so each conv chunk is ONE K=128
    matmul instead of 4 accumulating K=32 matmuls.
  - gram (small-branch) matmuls interleaved into half A as PE bubble filler.
  - two half-phases over i so half A's tail overlaps half B's S/exp/AV.
  - denominator broadcast via DVE stream_shuffle; tail elementwise on GPSIMD.
"""

import os
import sys

for _p in ("/opt/trn_rl_repo", "/root/.axon_site/_ro/trn_rl_repo"):
    if os.path.isdir(_p) and _p not in sys.path:
        sys.path.insert(0, _p)
        break

import ml_dtypes
import numpy as np

import concourse.bacc as bacc
import concourse.bass as bass
import concourse.mybir as mybir
import concourse.tile as tile
from concourse import bass_utils

B, H, W, D, C = 4, 16, 16, 16, 32
N = H * W * D            # 4096
NL = N // 2              # 2048 rows per core
DO = D - 3               # 13 conv output positions
NCORES = 8
NJC = N // 128           # 32 j-chunks

f32 = mybir.dt.float32
f32r = mybir.dt.float32r
bf16 = mybir.dt.bfloat16
i16 = mybir.dt.int16
FT = mybir.ActivationFunctionType
ALU = mybir.AluOpType
PSUM = bass.MemorySpace.PSUM

# Schraudolph bf16 exp on DVE: i16(round(x*A16 + B16)) bitcast bf16 ~ exp(x).
LN2 = 0.6931471805599453
A16 = 128.0 / LN2
B16 = 127.0 * 128.0 - 4.46   # magic-c correction balances error to ~±3.5%

N_DVE_PER_HALF = 14          # of 32 exp tiles per half on DVE (rest ACT)

# packed-constant layouts
PK_WQ, PK_WK, PK_SMALL = 0, 32, 64      # f32 pack: wq/wk rows 0:33, smalls
PKF = 68
PR_WVT, PR_ID32 = 0, 33                 # f32r pack
PRF = 65
# bf16 pack: stacked im2col weights [128,32] + flat k-major forms [32,128]
PB_WCH, PB_WPOS, PB_WCH4, PB_WPOS4 = 0, 32, 64, 192
PBF = 320


def _dve_pattern(n_tiles, n_dve):
    out, acc = [], 0
    for _ in range(n_tiles):
        acc += n_dve
        if acc >= n_tiles:
            acc -= n_tiles
            out.append(True)
        else:
            out.append(False)
    return out


def _emit(tc, nc, t, out_d):
    with (
        tc.tile_pool(name="const", bufs=1) as cp,
        tc.tile_pool(name="work", bufs=1) as wp,
    ):
        # ---- SBUF tiles ----
        qTP_r = cp.tile([128, N], bf16)        # q^T replicated x4 (host-side)
        qTloc_r = cp.tile([C + 1, NL], f32r)   # local q_aug^T (f32 bits)
        qc2_b = cp.tile([128, NJC, 128], bf16)  # [data|ones] x2 AV weights
        qc_f = cp.tile([128, NJC, C + 1], f32)  # gram operand
        pk = cp.tile([128, PKF], f32)
        pkr = cp.tile([C, PRF], f32r)
        pkb = cp.tile([128, PBF], bf16)

        qTloc_f = qTloc_r[0:C, 0:NL].bitcast(f32)
        wq_f = pk[0:C + 1, PK_WQ:PK_WQ + C]
        wk_f = pk[0:C + 1, PK_WK:PK_WK + C]
        bch_v = pk[0:C, PK_SMALL:PK_SMALL + 1]
        bpos_v = pk[0:C, PK_SMALL + 1:PK_SMALL + 2]
        gamma_v = pk[0:C, PK_SMALL + 2:PK_SMALL + 3]
        beta128_v = pk[:, PK_SMALL + 3:PK_SMALL + 4]
        beta_v = pk[0:1, PK_SMALL + 3:PK_SMALL + 4]
        wvT_r = pkr[0:C, PR_WVT:PR_WVT + C + 1]
        id32_r = pkr[0:C, PR_ID32:PR_ID32 + C]
        wch_st = pkb[:, PB_WCH:PB_WCH + C]
        wpos_st = pkb[:, PB_WPOS:PB_WPOS + C]
        wch4_flat = pkb[0:C, PB_WCH4:PB_WCH4 + 4 * C]
        wpos4_flat = pkb[0:C, PB_WPOS4:PB_WPOS4 + 4 * C]

        # ---- input DMAs spread across queues ----
        nc.sync.dma_start(qc_f[:, 0:8, :], t["qcf"][:, 0:8, :])
        nc.gpsimd.dma_start(qTP_r[0:C, :], t["qT"])
        nc.scalar.dma_start(pk[:], t["pk"])
        nc.scalar.dma_start(pkr[:], t["pkr"])
        nc.scalar.dma_start(pkb[:], t["pkb"])
        nc.sync.dma_start(qc_f[:, 8:NJC, :], t["qcf"][:, 8:NJC, :])
        nc.sync.dma_start(qTloc_r[:], t["qTloc"])
        nc.gpsimd.dma_start(qc2_b[:, :, 0:C], t["qc2d"])
        # trigger the ACT exp table load immediately (~1.3us)
        warm = wp.tile([1, 1], f32)
        nc.scalar.activation(warm[:], beta_v, FT.Exp)
        # replicate q^T onto row-groups 1..3 for the rolling S matmuls
        nc.vector.tensor_copy(qTP_r[C:2 * C, :], qTP_r[0:C, :])
        nc.vector.tensor_copy(qTP_r[2 * C:4 * C, :], qTP_r[0:2 * C, :])
        # qc2 = [data | ones] replicated onto both 64-column halves
        nc.vector.memset(qc2_b[:, :, C:2 * C], 1.0)
        nc.vector.tensor_copy(qc2_b[:, :, 2 * C:4 * C], qc2_b[:, :, 0:2 * C])

        relu_pos = wp.tile([C, NL], f32)
        relu_ch = wp.tile([C, NL], f32)
        sumT = wp.tile([C, NL], f32)
        paF_r = wp.tile([128, NL + 4], bf16)
        caF_r = wp.tile([128, NL + 4], bf16)
        tmp_ca = wp.tile([C, NL], f32)
        out_v = out_d.rearrange("(g kk r) f -> g r kk f", kk=16, r=C)

        # zero the conv-window pads: cols NL..NL+4 (block tail) and the
        # half-A/half-B seam cols 1024..1028 of caF (half A's shifted
        # replicas read 3 cols into not-yet-written half-B territory).
        nc.vector.memset(paF_r[:, NL:NL + 4], 0.0)
        nc.vector.memset(caF_r[:, NL:NL + 4], 0.0)
        nc.vector.memset(caF_r[:, NL // 2:NL // 2 + 4], 0.0)

        with tc.tile_pool(name="psAV", bufs=1, space=PSUM) as psAV:
            av0 = psAV.tile([128, 512], f32, tag="avA")  # half A: slices 0,1
            # gram quadrants: col-group c accumulates jc = c (mod 4); the four
            # [C, C] partial sums stack on partition groups of one psum bank.
            g_ps = psAV.tile([128, C], f32, tag="g")

            def emit_gram(j0, j1):
                for jc in range(j0, j1):
                    cq = jc & 3
                    nc.tensor.matmul(
                        g_ps[32 * cq:32 * cq + C, :],
                        qc_f[:, jc, 0:C], qc_f[:, jc, 0:C],
                        start=(jc < 4), stop=(jc >= NJC - 4),
                        tile_position=(0, 32 * cq), skip_group_check=True,
                    )

            def emit_small1():
                """G -> attn2 -> wpa (PE + tiny DVE/ACT ops)."""
                # G core: sum the 4 quadrant partials; aug row/col from a DVE
                # reduction of q^T (sum over all j); corner = N.
                g_sb = wp.tile([C + 1, C + 1], f32)
                gq1 = wp.tile([C, C], f32)
                gq2 = wp.tile([C, C], f32)
                gq3 = wp.tile([C, C], f32)
                nc.vector.tensor_copy(gq1[:], g_ps[C:2 * C, :])
                nc.vector.tensor_copy(gq2[:], g_ps[2 * C:3 * C, :])
                nc.vector.tensor_copy(gq3[:], g_ps[3 * C:4 * C, :])
                nc.vector.tensor_add(gq1[:], g_ps[0:C, :], gq1[:])
                nc.vector.tensor_add(gq2[:], gq2[:], gq3[:])
                nc.vector.tensor_add(g_sb[0:C, 0:C], gq1[:], gq2[:])
                csum = wp.tile([C, C], f32)
                nc.vector.memset(csum[:], 0.0)
                nc.vector.reduce_sum(csum[:, 0:1], qTP_r[0:C, :],
                                     axis=mybir.AxisListType.X)
                nc.vector.tensor_copy(g_sb[0:C, C:C + 1], csum[:, 0:1])
                csumT = wp.tile([C, C], f32)
                nc.vector.transpose(csumT[:], csum[:])
                nc.vector.tensor_copy(g_sb[C:C + 1, 0:C], csumT[0:1, :])
                nc.vector.memset(g_sb[C:C + 1, C:C + 1], float(N))
                t1_ps = psAV.tile([C + 1, C], f32, tag="g")
                nc.tensor.matmul(t1_ps[:], g_sb[:], wk_f, start=True, stop=True)
                t1_sb = wp.tile([C + 1, C], f32)
                nc.vector.tensor_copy(t1_sb[:], t1_ps[:])
                e2_ps = psAV.tile([C, C], f32, tag="g")
                nc.tensor.matmul(e2_ps[:], wq_f, t1_sb[:], start=True, stop=True)
                mx = wp.tile([C, 1], f32)
                nc.vector.reduce_max(mx[:], e2_ps[:], axis=mybir.AxisListType.X)
                nmx = wp.tile([C, 1], f32)
                nc.vector.tensor_scalar_mul(nmx[:], mx[:], -1.0)
                a_sb = wp.tile([C, C], f32)
                nc.scalar.activation(a_sb[:], e2_ps[:], FT.Exp, bias=nmx[:])
                sm = wp.tile([C, 1], f32)
                nc.vector.reduce_sum(sm[:], a_sb[:], axis=mybir.AxisListType.X)
                rc = wp.tile([C, 1], f32)
                nc.vector.reciprocal(rc[:], sm[:])
                a_n = wp.tile([C, C], f32r)
                nc.vector.tensor_scalar_mul(a_n[:], a_sb[:], rc[:])
                at_ps = psAV.tile([C, C], f32, tag="g")
                nc.tensor.matmul(at_ps[:], a_n[:], id32_r, start=True, stop=True)
                at_r = wp.tile([C, C], f32r)
                nc.vector.tensor_copy(at_r[:], at_ps[:])
                wpa_ps = psAV.tile([C + 1, C], f32, tag="g")
                nc.tensor.matmul(wpa_ps[:], wvT_r, at_r[:], start=True, stop=True)
                wpa_r = wp.tile([C + 1, C], f32r)
                nc.vector.tensor_copy(wpa_r[:], wpa_ps[:])
                return wpa_r

            def emit_small2(wpa_r):
                """pa branch + shifted replicas of paF."""
                for g in range(4):
                    pa_ps = psAV.tile([C, 512], f32, tag="g")
                    nc.tensor.matmul(
                        pa_ps[:], wpa_r[:], qTloc_r[:, g * 512:(g + 1) * 512],
                        start=True, stop=True,
                    )
                    nc.vector.scalar_tensor_tensor(
                        paF_r[0:C, g * 512:(g + 1) * 512], pa_ps[:], gamma_v,
                        qTloc_f[:, g * 512:(g + 1) * 512],
                        op0=ALU.mult, op1=ALU.add,
                    )
                for k in (1, 2, 3):
                    eng = (nc.gpsimd, nc.sync, nc.scalar)[k - 1]
                    eng.dma_start(
                        paF_r[32 * k:32 * k + C, 0:NL],
                        paF_r[0:C, k:k + NL],
                    )

            # ================= big branch: two half-phases =================
            s_roll = [0]   # rolling PE row-group for S matmuls

            def emit_av(av_t, jc, pt_ap):
                for p in range(2):
                    nc.tensor.matmul(
                        av_t[64 * p:64 * p + 64, :],
                        qc2_b[:, jc, 64 * p:64 * p + 64],
                        pt_ap[:, 512 * p:512 * p + 512],
                        start=(jc == 0), stop=(jc == NJC - 1),
                        tile_position=(0, 64 * p), skip_group_check=True,
                    )

            def emit_half(h, av_t, psS, ptp_a, ptp_d, dve_tiles, hooks):
                """S + exp + AV for slices (2h, 2h+1). One tile per jc; AV
                for jc is emitted after tile jc+1's exp (PE slack). hooks
                fire after the given tile index."""
                hooks = dict(hooks)
                pt_tiles = {}

                def s_tile(jc):
                    s_ps = psS.tile([128, 1024], f32, tag="s")
                    for s in range(2):
                        rp = s_roll[0] & 3
                        s_roll[0] += 1
                        cg = (2 * h + s) * 512
                        nc.tensor.matmul(
                            s_ps[:, s * 512:(s + 1) * 512],
                            qTP_r[32 * rp:32 * rp + C, jc * 128:(jc + 1) * 128],
                            qTP_r[32 * rp:32 * rp + C, cg:cg + 512],
                            start=True, stop=True,
                            tile_position=(32 * rp, 0), skip_group_check=True,
                        )
                    return s_ps

                def exp_tile(jc, s_ps):
                    if dve_tiles[jc]:
                        pti = ptp_d.tile([128, 1024], i16, tag="ptd")
                        nc.vector.tensor_scalar(
                            pti[:], s_ps[:], A16, B16, op0=ALU.mult, op1=ALU.add,
                        )
                        pt_tiles[jc] = pti[:].bitcast(bf16)
                    else:
                        ptt = ptp_a.tile([128, 1024], bf16, tag="pta")
                        nc.scalar.activation(ptt[:], s_ps[:], FT.Exp)
                        pt_tiles[jc] = ptt[:]

                # 2-tile batches: [S S S S | exp exp | AV AV AV AV] -- four S
                # matmuls burst through all 4 PE row-groups, AV lags 2 tiles.
                for t0 in range(0, NJC, 2):
                    sp0 = s_tile(t0)
                    sp1 = s_tile(t0 + 1)
                    exp_tile(t0, sp0)
                    exp_tile(t0 + 1, sp1)
                    if t0 >= 2:
                        emit_av(av_t, t0 - 2, pt_tiles.pop(t0 - 2))
                        emit_av(av_t, t0 - 1, pt_tiles.pop(t0 - 1))
                    if t0 in hooks:
                        hooks.pop(t0)()
                    if t0 + 1 in hooks:
                        hooks.pop(t0 + 1)()
                emit_av(av_t, NJC - 2, pt_tiles.pop(NJC - 2))
                emit_av(av_t, NJC - 1, pt_tiles.pop(NJC - 1))
                for k in sorted(hooks):
                    hooks.pop(k)()

            def emit_norm(h, av_t, add_eng, replicas=True):
                """caF[0:C, h*1024:+1024] = beta*ca/denom + q_loc, then the
                shifted replicas onto row-groups 1..3. The AV ones-rows
                (64p+32..64p+64) are already a 32-row denominator broadcast."""
                c0 = h * 1024
                din = wp.tile([64, 512], f32, tag=f"din{h}")
                nc.scalar.copy(din[0:C, :], av_t[C:2 * C, :])
                nc.scalar.copy(din[C:2 * C, :], av_t[3 * C:4 * C, :])
                rec = wp.tile([64, 512], f32, tag=f"rec{h}")
                nc.vector.reciprocal_approx_fast(rec[:], din[:])
                recB = wp.tile([64, 512], f32, tag=f"recB{h}")
                nc.vector.tensor_scalar_mul(recB[:], rec[:], beta128_v[0:64])
                for p in range(2):
                    nc.vector.tensor_mul(
                        tmp_ca[:, c0 + p * 512:c0 + (p + 1) * 512],
                        av_t[64 * p:64 * p + C, :],
                        recB[32 * p:32 * p + C, :],
                    )
                    add_eng.tensor_add(
                        caF_r[0:C, c0 + p * 512:c0 + (p + 1) * 512],
                        tmp_ca[:, c0 + p * 512:c0 + (p + 1) * 512],
                        qTloc_f[:, c0 + p * 512:c0 + (p + 1) * 512],
                    )
                if replicas:
                    for k in (1, 2, 3):
                        eng = (nc.gpsimd, nc.sync, nc.gpsimd)[k - 1]
                        eng.dma_start(
                            caF_r[32 * k:32 * k + C, c0:c0 + 1024],
                            caF_r[0:C, c0 + k:c0 + k + 1024],
                        )

            def conv_chunk(pool, tag, wst, x2t, bias_v, relu_out, cu,
                           kshift=None):
                """one [C, 512] conv chunk: a single K=128 im2col matmul on
                the shifted replicas, or (kshift) 4 accumulating K=32
                matmuls reading only row-group 0 (no replicas needed)."""
                cv = pool.tile([C, 512], f32, tag=tag)
                if kshift is None:
                    nc.tensor.matmul(
                        cv[:], wst, x2t[:, cu * 512:cu * 512 + 512],
                        start=True, stop=True,
                    )
                else:
                    for k in range(4):
                        nc.tensor.matmul(
                            cv[:],
                            kshift[:, k * C:(k + 1) * C],
                            x2t[0:C, cu * 512 + k:cu * 512 + k + 512],
                            start=(k == 0), stop=(k == 3),
                        )
                nc.scalar.activation(
                    relu_out[:, cu * 512:(cu + 1) * 512], cv[:],
                    FT.Relu, bias=bias_v,
                )

            def emit_convs(h, pool, tags, branches=("ch", "pos"), ks=False):
                i = 0
                for cu in (2 * h, 2 * h + 1):
                    if "ch" in branches:
                        conv_chunk(pool, tags[i % len(tags)], wch_st, caF_r,
                                   bch_v, relu_ch, cu,
                                   kshift=wch4_flat if ks else None)
                        i += 1
                    if "pos" in branches:
                        conv_chunk(pool, tags[i % len(tags)], wpos_st, paF_r,
                                   bpos_v, relu_pos, cu,
                                   kshift=wpos4_flat if ks else None)
                        i += 1

            def emit_outs(h, add_eng):
                for cu in (2 * h, 2 * h + 1):
                    add_eng.tensor_add(
                        sumT[:, cu * 512:(cu + 1) * 512],
                        relu_ch[:, cu * 512:(cu + 1) * 512],
                        relu_pos[:, cu * 512:(cu + 1) * 512],
                    )
                    tb = wp.tile([C, 512], f32, tag=f"ob{cu & 1}")
                    nc.vector.transpose(tb[:], sumT[:, cu * 512:(cu + 1) * 512])
                    eng = nc.sync if cu % 2 == 0 else nc.gpsimd
                    eng.dma_start(
                        out_v[cu],
                        tb[:].rearrange("r (kk f) -> r kk f", kk=16),
                    )

            patA = _dve_pattern(NJC, N_DVE_PER_HALF)
            patB = _dve_pattern(NJC, N_DVE_PER_HALF)
            wpa_box = {}
            with (
                tc.tile_pool(name="psS", bufs=3, space=PSUM) as psS,
                tc.tile_pool(name="ptpa", bufs=5) as ptp_a,
                tc.tile_pool(name="ptpd", bufs=5) as ptp_d,
            ):
                # gram: first chunks fill the head DMA wait; the rest are PE
                # bubble filler between half-A tiles.
                emit_gram(0, 8)
                hooksA = {
                    1: lambda: emit_gram(8, 14),
                    3: lambda: emit_gram(14, 20),
                    5: lambda: emit_gram(20, 26),
                    7: lambda: emit_gram(26, 32),
                    9: lambda: wpa_box.__setitem__("w", emit_small1()),
                    11: lambda: emit_small2(wpa_box["w"]),
                }
                emit_half(0, av0, psS, ptp_a, ptp_d, patA, hooksA)
                emit_norm(0, av0, nc.gpsimd)
                av1 = psAV.tile([128, 512], f32, tag="avA")
                hooksB = {
                    6: lambda: emit_convs(0, psAV, ["g", "g"]),
                    14: lambda: emit_outs(0, nc.gpsimd),
                    22: lambda: emit_convs(1, psAV, ["g", "g"],
                                           branches=("pos",)),
                }
                emit_half(1, av1, psS, ptp_a, ptp_d, patB, hooksB)
                emit_norm(1, av1, nc.vector, replicas=False)
            # psS closed: banks free for a pipelined final conv
            with tc.tile_pool(name="psC2", bufs=2, space=PSUM) as psC2:
                emit_convs(1, psC2, ["cv"], branches=("ch",), ks=True)
                emit_outs(1, nc.vector)


def _build():
    nc = bacc.Bacc("TRN2", target_bir_lowering=False, debug=False)
    t = {}

    def din(name, shape, dt):
        t[name] = nc.dram_tensor(name, shape, dt, kind="ExternalInput").ap()

    din("qT", [C, N], bf16)
    din("qTloc", [C + 1, NL], f32r)
    din("qc2d", [128, NJC, C], bf16)
    din("qcf", [128, NJC, C + 1], f32)
    din("pk", [128, PKF], f32)
    din("pkr", [C, PRF], f32r)
    din("pkb", [128, PBF], bf16)
    out_d = nc.dram_tensor("out", [NL, C], f32, kind="ExternalOutput").ap()

    with tile.TileContext(nc) as tc:
        _emit(tc, nc, t, out_d)
    nc.compile()
    return nc


_NC = None


def _get_nc():
    global _NC
    if _NC is None:
        _NC = _build()
    return _NC


def _prepare_in_maps(inputs):
    x = np.asarray(inputs["inputs"], np.float32)
    beta = np.asarray(inputs["beta"], np.float32)
    gamma = np.asarray(inputs["gamma"], np.float32)
    wq_aug = np.concatenate(
        [np.asarray(inputs["wq"], np.float32), np.asarray(inputs["bq"], np.float32)[None, :]], 0
    )
    wk_aug = np.concatenate(
        [np.asarray(inputs["wk"], np.float32), np.asarray(inputs["bk"], np.float32)[None, :]], 0
    )
    wv_aug = np.concatenate(
        [np.asarray(inputs["wv"], np.float32), np.asarray(inputs["bv"], np.float32)[None, :]], 0
    )
    pk = np.zeros((128, PKF), np.float32)
    pk[0:C + 1, PK_WQ:PK_WQ + C] = wq_aug
    pk[0:C + 1, PK_WK:PK_WK + C] = wk_aug
    pk[0:C, PK_SMALL] = np.asarray(inputs["b_ch"], np.float32)
    pk[0:C, PK_SMALL + 1] = np.asarray(inputs["b_pos"], np.float32)
    pk[0:C, PK_SMALL + 2] = gamma[0]
    pk[0:128, PK_SMALL + 3] = beta[0]
    pkr = np.zeros((C, PRF), np.float32)
    pkr[0:C, PR_WVT:PR_WVT + C + 1] = wv_aug.T
    pkr[0:C, PR_ID32:PR_ID32 + C] = np.eye(C, dtype=np.float32)
    pkb = np.zeros((128, PBF), np.float32)
    wch3 = np.asarray(inputs["w_ch"], np.float32).reshape(4, C, C)
    wpos3 = np.asarray(inputs["w_pos"], np.float32).reshape(4, C, C)
    pkb[:, PB_WCH:PB_WCH + C] = wch3.reshape(4 * C, C)
    pkb[:, PB_WPOS:PB_WPOS + C] = wpos3.reshape(4 * C, C)
    pkb[0:C, PB_WCH4:PB_WCH4 + 4 * C] = wch3.transpose(1, 0, 2).reshape(C, 4 * C)
    pkb[0:C, PB_WPOS4:PB_WPOS4 + 4 * C] = wpos3.transpose(1, 0, 2).reshape(C, 4 * C)
    pkb = pkb.astype(ml_dtypes.bfloat16)

    in_maps = []
    for core in range(NCORES):
        b, s = core // 2, core % 2
        qs = x[b].reshape(N, C)
        # local-half-first column permutation: S_T rhs slices [0, NL) are the
        # core's own rows; softmax sums over all j are order-invariant.
        q = np.concatenate([qs[s * NL:(s + 1) * NL], qs[(1 - s) * NL:(2 - s) * NL]])
        q_aug = np.concatenate([q, np.ones((N, 1), np.float32)], 1)
        qloc_aug = q_aug[:NL]
        qc = np.ascontiguousarray(q_aug.reshape(NJC, 128, C + 1).transpose(1, 0, 2))
        qT_b = np.ascontiguousarray(q.T).astype(ml_dtypes.bfloat16)
        m = {
            "qT": qT_b,
            "qTloc": np.ascontiguousarray(qloc_aug.T),
            "qc2d": np.ascontiguousarray(qc[:, :, :C]).astype(ml_dtypes.bfloat16),
            "qcf": qc,
            "pk": pk,
            "pkr": pkr,
            "pkb": pkb,
        }
        in_maps.append(m)
    return in_maps


def _run(inputs, trace=False):
    nc = _get_nc()
    in_maps = _prepare_in_maps(inputs)
    res = bass_utils.run_bass_kernel_spmd(
        nc, in_maps, core_ids=list(range(NCORES)), trace=trace
    )
    out = np.empty((B, H, W, DO, C), np.float32)
    for core in range(NCORES):
        b, s = core // 2, core % 2
        full = res.results[core]["out"].reshape(8, W, D, C)
        out[b, s * 8:(s + 1) * 8] = full[:, :, :DO, :]
    return out, res


def kernel(**inputs):
    out, _ = _run(inputs, trace=False)
    return out


# revision 24
# speedup vs baseline: 1.6139x; 1.0118x over previous
"""Trainium2 Bass kernel for nn_Attention_Embedding (spatial NxN attention +
channel CxC attention + conv3d(1,1,4) embedding head).

Sharding: 8 cores = 4 samples x 2 halves (split on H). Each core holds its
sample's full q (softmax rows are complete) and produces its own slice of the
final output; no cross-core communication.

v6 (~101us traced vs 148us traced baseline; kernel is PE-bound at the
HAM-throttled 1.2GHz clock with ~170-300ns fixed cost per matmul):
  - exp split across ACT (exact, bf16 out) and DVE (Schraudolph bit-trick:
    i16(round(x*128/ln2 + B16)) bitcast to bf16, ~±3.5% per element; softmax
    num/denom share the values so the final error stays ~2e-3) -- separate
    tile pools per engine so the two exp streams don't serialize.
  - 2-tile batches [S x4 | exp x2 | AV x4]: four S matmuls burst through all
    4 PE row-groups, AV lags 2 tiles so the PE never waits on exp.
  - psS bufs=3 ([128,1024] x 2 banks); both halves' AV accumulators share
    one psum slot (half B allocates after half A's normalize reads it).
  - conv3d via im2col on SHIFTED row-group replicas (replica k holds
    x[c,n+k]): one K=128 matmul per 512-chunk; the final half instead uses
    4 accumulating K=32 matmuls on row-group 0 (no replica DMAs on the
    critical tail).
  - gram as 4-way col-tiled [C,C] quadrant partials (M=32), interleaved into
    half A as PE bubble filler; aug row/col assembled from a DVE reduction.
  - two half-phases over i so half A's normalize/conv/relu/transpose/DMA
    overlaps half B's S/exp/AV stream.
  - the AV ones-rows are a ready-made 32-row denominator broadcast: the
    normalize chain is ACT-copy + recip + scale + mul + add, no shuffle.
  - constants packed into 3 DMAs; qTloc_f is a bitcast view of the f32r
    tile; elementwise tail work on GPSIMD/DVE by phase.
"""

import os
import sys

for _p in ("/opt/trn_rl_repo", "/root/.axon_site/_ro/trn_rl_repo"):
    if os.path.isdir(_p) and _p not in sys.path:
        sys.path.insert(0, _p)
        break

import ml_dtypes
import numpy as np

import concourse.bacc as bacc
import concourse.bass as bass
import concourse.mybir as mybir
import concourse.tile as tile
from concourse import bass_utils

B, H, W, D, C = 4, 16, 16, 16, 32
N = H * W * D            # 4096
NL = N // 2              # 2048 rows per core
DO = D - 3               # 13 conv output positions
NCORES = 8
NJC = N // 128           # 32 j-chunks

f32 = mybir.dt.float32
f32r = mybir.dt.float32r
bf16 = mybir.dt.bfloat16
i16 = mybir.dt.int16
FT = mybir.ActivationFunctionType
ALU = mybir.AluOpType
PSUM = bass.MemorySpace.PSUM

# Schraudolph bf16 exp on DVE: i16(round(x*A16 + B16)) bitcast bf16 ~ exp(x).
LN2 = 0.6931471805599453
A16 = 128.0 / LN2
B16 = 127.0 * 128.0 - 4.46   # magic-c correction balances error to ~±3.5%

N_DVE_PER_HALF = 13          # of 32 exp tiles per half on DVE (rest ACT)

# packed-constant layouts
PK_WQ, PK_WK, PK_SMALL = 0, 32, 64      # f32 pack: wq/wk rows 0:33, smalls
PKF = 68
PR_WVT, PR_ID32 = 0, 33                 # f32r pack
PRF = 65
# bf16 pack: stacked im2col weights [128,32] + flat k-major forms [32,128]
PB_WCH, PB_WPOS, PB_WCH4, PB_WPOS4 = 0, 32, 64, 192
PBF = 320


def _dve_pattern(n_tiles, n_dve):
    out, acc = [], 0
    for _ in range(n_tiles):
        acc += n_dve
        if acc >= n_tiles:
            acc -= n_tiles
            out.append(True)
        else:
            out.append(False)
    return out


def _emit(tc, nc, t, out_d):
    with (
        tc.tile_pool(name="const", bufs=1) as cp,
        tc.tile_pool(name="work", bufs=1) as wp,
    ):
        # ---- SBUF tiles ----
        qTP_r = cp.tile([128, N], bf16)        # q^T replicated x4 (host-side)
        qTloc_r = cp.tile([C + 1, NL], f32r)   # local q_aug^T (f32 bits)
        qc2_b = cp.tile([128, NJC, 128], bf16)  # [data|ones] x2 AV weights
        qc_f = cp.tile([128, NJC, C + 1], f32)  # gram operand
        pk = cp.tile([128, PKF], f32)
        pkr = cp.tile([C, PRF], f32r)
        pkb = cp.tile([128, PBF], bf16)

        qTloc_f = qTloc_r[0:C, 0:NL].bitcast(f32)
        wq_f = pk[0:C + 1, PK_WQ:PK_WQ + C]
        wk_f = pk[0:C + 1, PK_WK:PK_WK + C]
        bch_v = pk[0:C, PK_SMALL:PK_SMALL + 1]
        bpos_v = pk[0:C, PK_SMALL + 1:PK_SMALL + 2]
        gamma_v = pk[0:C, PK_SMALL + 2:PK_SMALL + 3]
        beta128_v = pk[:, PK_SMALL + 3:PK_SMALL + 4]
        beta_v = pk[0:1, PK_SMALL + 3:PK_SMALL + 4]
        wvT_r = pkr[0:C, PR_WVT:PR_WVT + C + 1]
        id32_r = pkr[0:C, PR_ID32:PR_ID32 + C]
        wch_st = pkb[:, PB_WCH:PB_WCH + C]
        wpos_st = pkb[:, PB_WPOS:PB_WPOS + C]
        wch4_flat = pkb[0:C, PB_WCH4:PB_WCH4 + 4 * C]
        wpos4_flat = pkb[0:C, PB_WPOS4:PB_WPOS4 + 4 * C]

        # ---- input DMAs spread across queues ----
        nc.sync.dma_start(qc_f[:, 0:8, :], t["qcf"][:, 0:8, :])
        nc.gpsimd.dma_start(qTP_r[0:C, :], t["qT"])
        nc.scalar.dma_start(pk[:], t["pk"])
        nc.scalar.dma_start(pkr[:], t["pkr"])
        nc.scalar.dma_start(pkb[:], t["pkb"])
        nc.sync.dma_start(qc_f[:, 8:NJC, :], t["qcf"][:, 8:NJC, :])
        nc.sync.dma_start(qTloc_r[:], t["qTloc"])
        nc.gpsimd.dma_start(qc2_b[:, :, 0:C], t["qc2d"])
        # trigger the ACT exp table load immediately (~1.3us)
        warm = wp.tile([1, 1], f32)
        nc.scalar.activation(warm[:], beta_v, FT.Exp)
        # replicate q^T onto row-groups 1..3 for the rolling S matmuls
        nc.vector.tensor_copy(qTP_r[C:2 * C, :], qTP_r[0:C, :])
        nc.vector.tensor_copy(qTP_r[2 * C:4 * C, :], qTP_r[0:2 * C, :])
        # qc2 = [data | ones] replicated onto both 64-column halves
        nc.vector.memset(qc2_b[:, :, C:2 * C], 1.0)
        nc.vector.tensor_copy(qc2_b[:, :, 2 * C:4 * C], qc2_b[:, :, 0:2 * C])

        relu_pos = wp.tile([C, NL], f32)
        relu_ch = wp.tile([C, NL], f32)
        sumT = wp.tile([C, NL], f32)
        paF_r = wp.tile([128, NL + 4], bf16)
        caF_r = wp.tile([128, NL + 4], bf16)
        tmp_ca = wp.tile([C, NL], f32)
        out_v = out_d.rearrange("(g kk r) f -> g r kk f", kk=16, r=C)

        # zero the conv-window pads: cols NL..NL+4 (block tail) and the
        # half-A/half-B seam cols 1024..1028 of caF (half A's shifted
        # replicas read 3 cols into not-yet-written half-B territory).
        nc.vector.memset(paF_r[:, NL:NL + 4], 0.0)
        nc.vector.memset(caF_r[:, NL:NL + 4], 0.0)
        nc.vector.memset(caF_r[:, NL // 2:NL // 2 + 4], 0.0)

        with tc.tile_pool(name="psAV", bufs=1, space=PSUM) as psAV:
            av0 = psAV.tile([128, 512], f32, tag="avA")  # half A: slices 0,1
            # gram quadrants: col-group c accumulates jc = c (mod 4); the four
            # [C, C] partial sums stack on partition groups of one psum bank.
            g_ps = psAV.tile([128, C], f32, tag="g")

            def emit_gram(j0, j1):
                for jc in range(j0, j1):
                    cq = jc & 3
                    nc.tensor.matmul(
                        g_ps[32 * cq:32 * cq + C, :],
                        qc_f[:, jc, 0:C], qc_f[:, jc, 0:C],
                        start=(jc < 4), stop=(jc >= NJC - 4),
                        tile_position=(0, 32 * cq), skip_group_check=True,
                    )

            def emit_small1():
                """G -> attn2 -> wpa (PE + tiny DVE/ACT ops)."""
                # G core: sum the 4 quadrant partials; aug row/col from a DVE
                # reduction of q^T (sum over all j); corner = N.
                g_sb = wp.tile([C + 1, C + 1], f32)
                gq1 = wp.tile([C, C], f32)
                gq2 = wp.tile([C, C], f32)
                gq3 = wp.tile([C, C], f32)
                nc.vector.tensor_copy(gq1[:], g_ps[C:2 * C, :])
                nc.vector.tensor_copy(gq2[:], g_ps[2 * C:3 * C, :])
                nc.vector.tensor_copy(gq3[:], g_ps[3 * C:4 * C, :])
                nc.vector.tensor_add(gq1[:], g_ps[0:C, :], gq1[:])
                nc.vector.tensor_add(gq2[:], gq2[:], gq3[:])
                nc.vector.tensor_add(g_sb[0:C, 0:C], gq1[:], gq2[:])
                csum = wp.tile([C, C], f32)
                nc.vector.memset(csum[:], 0.0)
                nc.vector.reduce_sum(csum[:, 0:1], qTP_r[0:C, :],
                                     axis=mybir.AxisListType.X)
                nc.vector.tensor_copy(g_sb[0:C, C:C + 1], csum[:, 0:1])
                csumT = wp.tile([C, C], f32)
                nc.vector.transpose(csumT[:], csum[:])
                nc.vector.tensor_copy(g_sb[C:C + 1, 0:C], csumT[0:1, :])
                nc.vector.memset(g_sb[C:C + 1, C:C + 1], float(N))
                t1_ps = psAV.tile([C + 1, C], f32, tag="g")
                nc.tensor.matmul(t1_ps[:], g_sb[:], wk_f, start=True, stop=True)
                t1_sb = wp.tile([C + 1, C], f32)
                nc.vector.tensor_copy(t1_sb[:], t1_ps[:])
                e2_ps = psAV.tile([C, C], f32, tag="g")
                nc.tensor.matmul(e2_ps[:], wq_f, t1_sb[:], start=True, stop=True)
                mx = wp.tile([C, 1], f32)
                nc.vector.reduce_max(mx[:], e2_ps[:], axis=mybir.AxisListType.X)
                nmx = wp.tile([C, 1], f32)
                nc.vector.tensor_scalar_mul(nmx[:], mx[:], -1.0)
                a_sb = wp.tile([C, C], f32)
                nc.scalar.activation(a_sb[:], e2_ps[:], FT.Exp, bias=nmx[:])
                sm = wp.tile([C, 1], f32)
                nc.vector.reduce_sum(sm[:], a_sb[:], axis=mybir.AxisListType.X)
                rc = wp.tile([C, 1], f32)
                nc.vector.reciprocal(rc[:], sm[:])
                a_n = wp.tile([C, C], f32r)
                nc.vector.tensor_scalar_mul(a_n[:], a_sb[:], rc[:])
                at_ps = psAV.tile([C, C], f32, tag="g")
                nc.tensor.matmul(at_ps[:], a_n[:], id32_r, start=True, stop=True)
                at_r = wp.tile([C, C], f32r)
                nc.vector.tensor_copy(at_r[:], at_ps[:])
                wpa_ps = psAV.tile([C + 1, C], f32, tag="g")
                nc.tensor.matmul(wpa_ps[:], wvT_r, at_r[:], start=True, stop=True)
                wpa_r = wp.tile([C + 1, C], f32r)
                nc.vector.tensor_copy(wpa_r[:], wpa_ps[:])
                return wpa_r

            def emit_small2(wpa_r):
                """pa branch + shifted replicas of paF."""
                for g in range(4):
                    pa_ps = psAV.tile([C, 512], f32, tag="g")
                    nc.tensor.matmul(
                        pa_ps[:], wpa_r[:], qTloc_r[:, g * 512:(g + 1) * 512],
                        start=True, stop=True,
                    )
                    nc.vector.scalar_tensor_tensor(
                        paF_r[0:C, g * 512:(g + 1) * 512], pa_ps[:], gamma_v,
                        qTloc_f[:, g * 512:(g + 1) * 512],
                        op0=ALU.mult, op1=ALU.add,
                    )
                for k in (1, 2, 3):
                    eng = (nc.gpsimd, nc.sync, nc.scalar)[k - 1]
                    eng.dma_start(
                        paF_r[32 * k:32 * k + C, 0:NL],
                        paF_r[0:C, k:k + NL],
                    )

            # ================= big branch: two half-phases =================
            s_roll = [0]   # rolling PE row-group for S matmuls

            def emit_av(av_t, jc, pt_ap):
                for p in range(2):
                    nc.tensor.matmul(
                        av_t[64 * p:64 * p + 64, :],
                        qc2_b[:, jc, 64 * p:64 * p + 64],
                        pt_ap[:, 512 * p:512 * p + 512],
                        start=(jc == 0), stop=(jc == NJC - 1),
                        tile_position=(0, 64 * p), skip_group_check=True,
                    )

            def emit_half(h, av_t, psS, ptp_a, ptp_d, dve_tiles, hooks):
                """S + exp + AV for slices (2h, 2h+1). One tile per jc; AV
                for jc is emitted after tile jc+1's exp (PE slack). hooks
                fire after the given tile index."""
                hooks = dict(hooks)
                pt_tiles = {}

                def s_tile(jc):
                    s_ps = psS.tile([128, 1024], f32, tag="s")
                    for s in range(2):
                        rp = s_roll[0] & 3
                        s_roll[0] += 1
                        cg = (2 * h + s) * 512
                        nc.tensor.matmul(
                            s_ps[:, s * 512:(s + 1) * 512],
                            qTP_r[32 * rp:32 * rp + C, jc * 128:(jc + 1) * 128],
                            qTP_r[32 * rp:32 * rp + C, cg:cg + 512],
                            start=True, stop=True,
                            tile_position=(32 * rp, 0), skip_group_check=True,
                        )
                    return s_ps

                def exp_tile(jc, s_ps):
                    if dve_tiles[jc]:
                        pti = ptp_d.tile([128, 1024], i16, tag="ptd")
                        nc.vector.tensor_scalar(
                            pti[:], s_ps[:], A16, B16, op0=ALU.mult, op1=ALU.add,
                        )
                        pt_tiles[jc] = pti[:].bitcast(bf16)
                    else:
                        ptt = ptp_a.tile([128, 1024], bf16, tag="pta")
                        nc.scalar.activation(ptt[:], s_ps[:], FT.Exp)
                        pt_tiles[jc] = ptt[:]

                # 2-tile batches: [S S S S | exp exp | AV AV AV AV] -- four S
                # matmuls burst through all 4 PE row-groups, AV lags 2 tiles.
                for t0 in range(0, NJC, 2):
                    sp0 = s_tile(t0)
                    sp1 = s_tile(t0 + 1)
                    exp_tile(t0, sp0)
                    exp_tile(t0 + 1, sp1)
                    if t0 >= 2:
                        emit_av(av_t, t0 - 2, pt_tiles.pop(t0 - 2))
                        emit_av(av_t, t0 - 1, pt_tiles.pop(t0 - 1))
                    if t0 in hooks:
                        hooks.pop(t0)()
                    if t0 + 1 in hooks:
                        hooks.pop(t0 + 1)()
                emit_av(av_t, NJC - 2, pt_tiles.pop(NJC - 2))
                emit_av(av_t, NJC - 1, pt_tiles.pop(NJC - 1))
                for k in sorted(hooks):
                    hooks.pop(k)()

            def emit_norm(h, av_t, add_eng, replicas=True):
                """caF[0:C, h*1024:+1024] = beta*ca/denom + q_loc, then the
                shifted replicas onto row-groups 1..3. The AV ones-rows
                (64p+32..64p+64) are already a 32-row denominator broadcast."""
                c0 = h * 1024
                din = wp.tile([64, 512], f32, tag=f"din{h}")
                nc.scalar.copy(din[0:C, :], av_t[C:2 * C, :])
                nc.scalar.copy(din[C:2 * C, :], av_t[3 * C:4 * C, :])
                rec = wp.tile([64, 512], f32, tag=f"rec{h}")
                nc.vector.reciprocal_approx_fast(rec[:], din[:])
                recB = wp.tile([64, 512], f32, tag=f"recB{h}")
                nc.vector.tensor_scalar_mul(recB[:], rec[:], beta128_v[0:64])
                for p in range(2):
                    nc.vector.tensor_mul(
                        tmp_ca[:, c0 + p * 512:c0 + (p + 1) * 512],
                        av_t[64 * p:64 * p + C, :],
                        recB[32 * p:32 * p + C, :],
                    )
                    add_eng.tensor_add(
                        caF_r[0:C, c0 + p * 512:c0 + (p + 1) * 512],
                        tmp_ca[:, c0 + p * 512:c0 + (p + 1) * 512],
                        qTloc_f[:, c0 + p * 512:c0 + (p + 1) * 512],
                    )
                if replicas:
                    for k in (1, 2, 3):
                        eng = (nc.gpsimd, nc.sync, nc.gpsimd)[k - 1]
                        eng.dma_start(
                            caF_r[32 * k:32 * k + C, c0:c0 + 1024],
                            caF_r[0:C, c0 + k:c0 + k + 1024],
                        )

            def conv_chunk(pool, tag, wst, x2t, bias_v, relu_out, cu,
                           kshift=None):
                """one [C, 512] conv chunk: a single K=128 im2col matmul on
                the shifted replicas, or (kshift) 4 accumulating K=32
                matmuls reading only row-group 0 (no replicas needed)."""
                cv = pool.tile([C, 512], f32, tag=tag)
                if kshift is None:
                    nc.tensor.matmul(
                        cv[:], wst, x2t[:, cu * 512:cu * 512 + 512],
                        start=True, stop=True,
                    )
                else:
                    for k in range(4):
                        nc.tensor.matmul(
                            cv[:],
                            kshift[:, k * C:(k + 1) * C],
                            x2t[0:C, cu * 512 + k:cu * 512 + k + 512],
                            start=(k == 0), stop=(k == 3),
                        )
                nc.scalar.activation(
                    relu_out[:, cu * 512:(cu + 1) * 512], cv[:],
                    FT.Relu, bias=bias_v,
                )

            def emit_convs(h, pool, tags, branches=("ch", "pos"), ks=False):
                i = 0
                for cu in (2 * h, 2 * h + 1):
                    if "ch" in branches:
                        conv_chunk(pool, tags[i % len(tags)], wch_st, caF_r,
                                   bch_v, relu_ch, cu,
                                   kshift=wch4_flat if ks else None)
                        i += 1
                    if "pos" in branches:
                        conv_chunk(pool, tags[i % len(tags)], wpos_st, paF_r,
                                   bpos_v, relu_pos, cu,
                                   kshift=wpos4_flat if ks else None)
                        i += 1

            def emit_outs(h, add_eng):
                for cu in (2 * h, 2 * h + 1):
                    add_eng.tensor_add(
                        sumT[:, cu * 512:(cu + 1) * 512],
                        relu_ch[:, cu * 512:(cu + 1) * 512],
                        relu_pos[:, cu * 512:(cu + 1) * 512],
                    )
                    tb = wp.tile([C, 512], f32, tag=f"ob{cu & 1}")
                    nc.vector.transpose(tb[:], sumT[:, cu * 512:(cu + 1) * 512])
                    eng = nc.sync if cu % 2 == 0 else nc.gpsimd
                    eng.dma_start(
                        out_v[cu],
                        tb[:].rearrange("r (kk f) -> r kk f", kk=16),
                    )

            patA = _dve_pattern(NJC, N_DVE_PER_HALF)
            patB = _dve_pattern(NJC, N_DVE_PER_HALF)
            wpa_box = {}
            with (
                tc.tile_pool(name="psS", bufs=3, space=PSUM) as psS,
                tc.tile_pool(name="ptpa", bufs=5) as ptp_a,
                tc.tile_pool(name="ptpd", bufs=5) as ptp_d,
            ):
                # gram: first chunks fill the head DMA wait; the rest are PE
                # bubble filler between half-A tiles.
                emit_gram(0, 8)
                hooksA = {
                    1: lambda: emit_gram(8, 14),
                    3: lambda: emit_gram(14, 20),
                    5: lambda: emit_gram(20, 26),
                    7: lambda: emit_gram(26, 32),
                    9: lambda: wpa_box.__setitem__("w", emit_small1()),
                    11: lambda: emit_small2(wpa_box["w"]),
                }
                emit_half(0, av0, psS, ptp_a, ptp_d, patA, hooksA)
                emit_norm(0, av0, nc.gpsimd)
                av1 = psAV.tile([128, 512], f32, tag="avA")
                hooksB = {
                    6: lambda: emit_convs(0, psAV, ["g", "g"]),
                    14: lambda: emit_outs(0, nc.gpsimd),
                    22: lambda: emit_convs(1, psAV, ["g", "g"],
                                           branches=("pos",)),
                }
                emit_half(1, av1, psS, ptp_a, ptp_d, patB, hooksB)
                emit_norm(1, av1, nc.vector, replicas=False)
            # psS closed: banks free for a pipelined final conv
            with tc.tile_pool(name="psC2", bufs=2, space=PSUM) as psC2:
                emit_convs(1, psC2, ["cv"], branches=("ch",), ks=True)
                emit_outs(1, nc.vector)


def _build():
    nc = bacc.Bacc("TRN2", target_bir_lowering=False, debug=False)
    t = {}

    def din(name, shape, dt):
        t[name] = nc.dram_tensor(name, shape, dt, kind="ExternalInput").ap()

    din("qT", [C, N], bf16)
    din("qTloc", [C + 1, NL], f32r)
    din("qc2d", [128, NJC, C], bf16)
    din("qcf", [128, NJC, C + 1], f32)
    din("pk", [128, PKF], f32)
    din("pkr", [C, PRF], f32r)
    din("pkb", [128, PBF], bf16)
    out_d = nc.dram_tensor("out", [NL, C], f32, kind="ExternalOutput").ap()

    with tile.TileContext(nc) as tc:
        _emit(tc, nc, t, out_d)
    nc.compile()
    return nc


_NC = None


def _get_nc():
    global _NC
    if _NC is None:
        _NC = _build()
    return _NC


def _prepare_in_maps(inputs):
    x = np.asarray(inputs["inputs"], np.float32)
    beta = np.asarray(inputs["beta"], np.float32)
    gamma = np.asarray(inputs["gamma"], np.float32)
    wq_aug = np.concatenate(
        [np.asarray(inputs["wq"], np.float32), np.asarray(inputs["bq"], np.float32)[None, :]], 0
    )
    wk_aug = np.concatenate(
        [np.asarray(inputs["wk"], np.float32), np.asarray(inputs["bk"], np.float32)[None, :]], 0
    )
    wv_aug = np.concatenate(
        [np.asarray(inputs["wv"], np.float32), np.asarray(inputs["bv"], np.float32)[None, :]], 0
    )
    pk = np.zeros((128, PKF), np.float32)
    pk[0:C + 1, PK_WQ:PK_WQ + C] = wq_aug
    pk[0:C + 1, PK_WK:PK_WK + C] = wk_aug
    pk[0:C, PK_SMALL] = np.asarray(inputs["b_ch"], np.float32)
    pk[0:C, PK_SMALL + 1] = np.asarray(inputs["b_pos"], np.float32)
    pk[0:C, PK_SMALL + 2] = gamma[0]
    pk[0:128, PK_SMALL + 3] = beta[0]
    pkr = np.zeros((C, PRF), np.float32)
    pkr[0:C, PR_WVT:PR_WVT + C + 1] = wv_aug.T
    pkr[0:C, PR_ID32:PR_ID32 + C] = np.eye(C, dtype=np.float32)
    pkb = np.zeros((128, PBF), np.float32)
    wch3 = np.asarray(inputs["w_ch"], np.float32).reshape(4, C, C)
    wpos3 = np.asarray(inputs["w_pos"], np.float32).reshape(4, C, C)
    pkb[:, PB_WCH:PB_WCH + C] = wch3.reshape(4 * C, C)
    pkb[:, PB_WPOS:PB_WPOS + C] = wpos3.reshape(4 * C, C)
    pkb[0:C, PB_WCH4:PB_WCH4 + 4 * C] = wch3.transpose(1, 0, 2).reshape(C, 4 * C)
    pkb[0:C, PB_WPOS4:PB_WPOS4 + 4 * C] = wpos3.transpose(1, 0, 2).reshape(C, 4 * C)
    pkb = pkb.astype(ml_dtypes.bfloat16)

    in_maps = []
    for core in range(NCORES):
        b, s = core // 2, core % 2
        qs = x[b].reshape(N, C)
        # local-half-first column permutation: S_T rhs slices [0, NL) are the
        # core's own rows; softmax sums over all j are order-invariant.
        q = np.concatenate([qs[s * NL:(s + 1) * NL], qs[(1 - s) * NL:(2 - s) * NL]])
        q_aug = np.concatenate([q, np.ones((N, 1), np.float32)], 1)
        qloc_aug = q_aug[:NL]
        qc = np.ascontiguousarray(q_aug.reshape(NJC, 128, C + 1).transpose(1, 0, 2))
        qT_b = np.ascontiguousarray(q.T).astype(ml_dtypes.bfloat16)
        m = {
            "qT": qT_b,
            "qTloc": np.ascontiguousarray(qloc_aug.T),
            "qc2d": np.ascontiguousarray(qc[:, :, :C]).astype(ml_dtypes.bfloat16),
            "qcf": qc,
            "pk": pk,
            "pkr": pkr,
            "pkb": pkb,
        }
        in_maps.append(m)
    return in_maps


def _run(inputs, trace=False):
    nc = _get_nc()
    in_maps = _prepare_in_maps(inputs)
    res = bass_utils.run_bass_kernel_spmd(
        nc, in_maps, core_ids=list(range(NCORES)), trace=trace
    )
    out = np.empty((B, H, W, DO, C), np.float32)
    for core in range(NCORES):
        b, s = core // 2, core % 2
        full = res.results[core]["out"].reshape(8, W, D, C)
        out[b, s * 8:(s + 1) * 8] = full[:, :, :DO, :]
    return out, res


def kernel(**inputs):
    out, _ = _run(inputs, trace=False)
    return out
